# revision 1
# baseline (speedup 1.0000x reference)
"""DiffPool GNN encoder on 8 Trainium2 NeuronCores.

Data-parallel over graphs: core d owns graphs [16d, 16d+16) = node rows
[1024d, 1024d+1024). Host builds each core's dense A^T slab (bf16 0/1),
the per-graph block-diagonal 64x64 A blocks, x^T, and pre-chunked bf16
weights. The device kernel computes, per core:

  level 0:  Y = x @ [pWh0|eWh0]            (row-major, lhsT = x^T)
            Mt = Y^T @ AT_slab             (feature-major aggregation)
            pool chain -> softmax S0 -> block-diag S_bd [1024,160]
            emb chain  -> Z [1024,300] row-major
            X1T = Z^T @ S_bd, A1 = S^T A S (+ transposed variant), both
            via block-diag matmuls
  level 1:  same, 16 graphs x 10 nodes
  level 2:  emb only (pool softmax over k=1 is identically 1), X3 = per
            graph column sum of Z2
  head:     out^T = lW2^T @ relu(lW1^T @ X3T + lb1) + lb2   [128, 16]

Host gathers the 8 [128,16] outputs into the [128,128] result.
"""

import numpy as np
import ml_dtypes

BF = ml_dtypes.bfloat16
F8 = ml_dtypes.float8_e4m3fn
N_CORES = 8
N_NODES = 8192
B = 128
GPC = 16          # graphs per core
R = 1024          # rows per core
N0 = 64           # nodes per graph at level 0
D = 300
K0, K1 = 10, 4
K1NODES = 10  # nodes per graph at level 1

_prog_cache = {}


def _patch_tile_drain():
    """This container's walrus rejects >2 sync waits on one instruction;
    split the Tile tail-drain waits across several drains."""
    import concourse.tile as tile_mod
    from concourse.vector_clock import ScopedClock, VectorClock

    if getattr(tile_mod.TileContext, "_drain_patched", False):
        return

    def _patched(self, tick_clock, wait_clock):
        gc = tick_clock.global_clock
        n = len(gc)
        for start in range(0, n, 2):
            partial = VectorClock(
                [gc[p] if start <= p < start + 2 else 0 for p in range(n)]
            )
            di = self.nc.sync.drain()
            wait_clock.add_sem_waits(di.ins, ScopedClock({None: partial}))
        self.nc.all_engine_barrier()
        assert self.sems is not None
        popped = self.nc._tile_sem_poison_stack.pop()
        assert popped is self._sem_poison
        self.nc.clear_and_free_semaphores(list(self.sems.allocated().values()))
        self.nc.all_engine_barrier()

    tile_mod.TileContext._drain_and_barrier = _patched
    tile_mod.TileContext._drain_patched = True


def _split_excess_waits(nc, max_waits=1):
    """walrus here rejects instructions with >2 sync waits. Move excess waits
    onto injected same-engine nops placed immediately before the instruction
    (engine queues execute in order, so this preserves semantics)."""
    import concourse.mybir as mybir

    blocks = nc.m.functions[0].blocks
    for b in blocks:
        idx = 0
        while idx < len(b.instructions):
            inst = b.instructions[idx]
            si = inst.sync_info
            lim = max_waits
            if si is None or not si.on_wait or len(si.on_wait) <= lim:
                idx += 1
                continue
            waits = list(si.on_wait)
            keep = waits[-lim:]
            rest = waits[:-lim]
            inst.sync_info = mybir.SyncInfo(
                on_wait=keep, on_update=list(si.on_update or []))
            nops = []
            for c0 in range(0, len(rest)):
                n = nc.engines[inst.engine].nop(nofuse=True)
                ni = n.ins
                ni.sync_info = mybir.SyncInfo(
                    on_wait=[rest[c0]], on_update=[])
                # remove from wherever the builder appended it
                for b2 in blocks:
                    for j in range(len(b2.instructions) - 1, -1, -1):
                        if b2.instructions[j] is ni:
                            b2.instructions.pop(j)
                            break
                nops.append(ni)
            for n_off, ni in enumerate(nops):
                b.instructions.insert(idx + n_off, ni)
            idx += len(nops) + 1


def _softmax_rowmajor(nc, pool, psum_l, out_sb, k):
    """Row-major softmax over free dim k. psum_l: [p, k] f32 logits;
    out_sb: [p, k] bf16 destination."""
    import concourse.mybir as mybir

    p = psum_l.shape[0]
    mx = pool.tile([p, 1], mybir.dt.float32, tag="smax_mx")
    sm = pool.tile([p, 1], mybir.dt.float32, tag="smax_sum")
    rc = pool.tile([p, 1], mybir.dt.float32, tag="smax_rcp")
    ex = pool.tile([p, k], mybir.dt.float32, tag="smax_exp")
    nc.vector.reduce_max(mx[:], psum_l[:], axis=mybir.AxisListType.X, negate=True)
    nc.scalar.activation(
        ex[:], psum_l[:], mybir.ActivationFunctionType.Exp,
        bias=mx[:], scale=1.0, accum_out=sm[:],
    )
    nc.vector.reciprocal(rc[:], sm[:])
    nc.vector.tensor_scalar_mul(out_sb[:], ex[:], rc[:])


def _build_program():
    import concourse.bass as bass
    import concourse.mybir as mybir
    import concourse.tile as tile

    _patch_tile_drain()
    f32 = mybir.dt.float32
    bf16 = mybir.dt.bfloat16
    fp8 = mybir.dt.float8e4

    nc = bass.Bass()

    # ---- DRAM inputs (per-core shards handed via in_maps) ----
    d_xT = nc.dram_tensor("xT", [8, 100, 3, 1024], bf16, kind="ExternalInput")
    d_at = nc.dram_tensor("at", [32, 128, 2, 1024], fp8, kind="ExternalInput")
    d_adiag = nc.dram_tensor("adiag", [128, 8, 128], bf16, kind="ExternalInput")
    d_adiagT = nc.dram_tensor("adiagT", [128, 8, 128], bf16, kind="ExternalInput")
    d_wcat0 = nc.dram_tensor("wcat0", [100, 3, 492], bf16, kind="ExternalInput")
    d_pWl0 = nc.dram_tensor("pWl0", [128, 2, 300], bf16, kind="ExternalInput")
    d_pWo0 = nc.dram_tensor("pWo0", [100, 3, K0], bf16, kind="ExternalInput")
    d_eWl0 = nc.dram_tensor("eWl0", [128, 3, 600], bf16, kind="ExternalInput")
    d_eWo0 = nc.dram_tensor("eWo0", [120, 5, 300], bf16, kind="ExternalInput")
    d_pWh1 = nc.dram_tensor("pWh1", [100, 3, 150], bf16, kind="ExternalInput")
    d_pWl1 = nc.dram_tensor("pWl1", [75, 2, 300], bf16, kind="ExternalInput")
    d_pWo1 = nc.dram_tensor("pWo1", [100, 3, K1], bf16, kind="ExternalInput")
    d_eWh1 = nc.dram_tensor("eWh1", [100, 3, 300], bf16, kind="ExternalInput")
    d_eWl1 = nc.dram_tensor("eWl1", [100, 3, 600], bf16, kind="ExternalInput")
    d_eWo1 = nc.dram_tensor("eWo1", [120, 5, 300], bf16, kind="ExternalInput")
    d_eWh2 = nc.dram_tensor("eWh2", [100, 3, 300], bf16, kind="ExternalInput")
    d_eWl2 = nc.dram_tensor("eWl2", [100, 3, 600], bf16, kind="ExternalInput")
    d_eWo2 = nc.dram_tensor("eWo2", [120, 5, 300], bf16, kind="ExternalInput")
    d_lW1 = nc.dram_tensor("lW1", [100, 3, 600], bf16, kind="ExternalInput")
    d_lW2 = nc.dram_tensor("lW2", [120, 5, 128], bf16, kind="ExternalInput")
    d_lb1 = nc.dram_tensor("lb1", [120, 5], f32, kind="ExternalInput")
    d_lb2 = nc.dram_tensor("lb2", [128, 1], f32, kind="ExternalInput")
    d_ones = nc.dram_tensor("ones16", [64, GPC], bf16, kind="ExternalInput")
    d_s1mask = nc.dram_tensor("s1mask", [80, 2, 64], bf16, kind="ExternalInput")
    d_out = nc.dram_tensor("out", [128, GPC], f32, kind="ExternalOutput")

    with tile.TileContext(nc) as tc:
        with (
            tc.tile_pool(name="wpool", bufs=1) as wp,      # resident weights
            tc.tile_pool(name="big", bufs=1) as bigp,      # resident activations
            tc.tile_pool(name="atp", bufs=6) as atp,       # streamed AT tiles
            tc.tile_pool(name="tmp", bufs=4) as tmp,       # small temporaries
            tc.tile_pool(name="ps", bufs=8, space="PSUM") as psC,
        ):
            def load(dram, shape, eng=None):
                t = wp.tile(shape, dram.dtype, tag=dram.name)
                (eng or nc.scalar).dma_start(t[:], dram[:])
                return t

            wcat0 = load(d_wcat0, [100, 3, 492])
            xT = wp.tile([100, 3, N_NODES], bf16, tag="xT")
            for nq in range(8):
                nc.sync.dma_start(xT[:, :, nq * 1024:(nq + 1) * 1024], d_xT[nq])
            pWl0 = load(d_pWl0, [128, 2, 300])
            pWo0 = load(d_pWo0, [100, 3, K0])
            eWl0 = load(d_eWl0, [128, 3, 600])
            eWo0 = load(d_eWo0, [120, 5, 300])
            adiag = load(d_adiag, [128, 8, 128])
            adiagT = load(d_adiagT, [128, 8, 128])
            pWh1 = load(d_pWh1, [100, 3, 150])
            pWl1 = load(d_pWl1, [75, 2, 300])
            pWo1 = load(d_pWo1, [100, 3, K1])
            eWh1 = load(d_eWh1, [100, 3, 300])
            eWl1 = load(d_eWl1, [100, 3, 600])
            eWo1 = load(d_eWo1, [120, 5, 300])
            eWh2 = load(d_eWh2, [100, 3, 300])
            eWl2 = load(d_eWl2, [100, 3, 600])
            eWo2 = load(d_eWo2, [120, 5, 300])
            lW1 = load(d_lW1, [100, 3, 600])
            lW2 = load(d_lW2, [120, 5, 128])
            lb1 = load(d_lb1, [120, 5])
            lb2 = load(d_lb2, [128, 1])
            ones16 = load(d_ones, [64, GPC])

            Relu = mybir.ActivationFunctionType.Relu


            # ---- stage A: Y[1024*8? no: 8192, 450] row-major, bf16 ----
            # Y[128m+p, f] = sum_d x[128m+p, d] * wcat[d, f]
            Y = bigp.tile([128, 64, 512], fp8, tag="Y")
            for m in range(64):
                ps = psC.tile([128, 492], f32, tag="ps", name="psY")
                for kc in range(3):
                    nc.tensor.matmul(
                        ps[:], xT[:, kc, m * 128:(m + 1) * 128],
                        wcat0[:, kc, :],
                        start=(kc == 0), stop=(kc == 2),
                    )
                nc.vector.tensor_copy(Y[:, m, 0:492], ps[:])

            # ---- stage B: Mt = Y^T @ AT  (feature-major), relu -> G ----
            # feat chunks: pool [0:75),[75:150)  emb [150:250),[250:350),[350:450)
            Gp0 = bigp.tile([128, 1024], bf16, tag="Gp0")
            Gp1 = bigp.tile([32, 1024], bf16, tag="Gp1")
            Ge0 = bigp.tile([64, 1024], bf16, tag="Ge0")
            Ge1 = bigp.tile([128, 1024], bf16, tag="Ge1")
            Ge2 = bigp.tile([108, 1024], bf16, tag="Ge2")
            mchunks = [(0, 128), (128, 128), (256, 128), (384, 108)]
            pss = [[psC.tile([mchunks[mi][1], 512], f32, tag="ps",
                            name=f"psB_{nb}_{mi}")
                    for mi in range(4)] for nb in range(2)]
            for kk in range(0, 64, 2):
                at_t = atp.tile([128, 2, 1024], fp8, tag="at")
                nc.sync.dma_start(at_t[:], d_at[kk // 2])
                for nb in range(2):
                    for mi, (off, sz) in enumerate(mchunks):
                        nc.tensor.matmul(
                            pss[nb][mi][:],
                            Y[:, kk:kk + 2, off:off + sz],
                            at_t[:, :, nb * 512:(nb + 1) * 512],
                            start=(kk == 0), stop=(kk == 62),
                            perf_mode=mybir.MatmulPerfMode.DoubleRow,
                        )
            for nb in range(2):
                nbs = slice(nb * 512, (nb + 1) * 512)
                nc.scalar.activation(Gp0[:, nbs], pss[nb][0][:], Relu)
                nc.scalar.activation(Gp1[:, nbs], pss[nb][1][0:32, :], Relu)
                nc.scalar.activation(Ge0[:, nbs], pss[nb][1][64:128, :], Relu)
                nc.scalar.activation(Ge1[:, nbs], pss[nb][2][:], Relu)
                nc.scalar.activation(Ge2[:, nbs], pss[nb][3][:], Relu)

            # ---- level-0 chains (pool/emb interleaved for PE density) ----
            H1p = bigp.tile([100, 3, 1024], bf16, tag="H1p")
            H1e = bigp.tile([120, 5, 1024], bf16, tag="H1e")
            S_bd = bigp.tile([128, 8, 160], bf16, tag="S_bd")
            nc.any.memzero(S_bd[:])
            for nb in range(2):
                nbs = slice(nb * 512, (nb + 1) * 512)
                for mc in range(5):
                    ps = psC.tile([120, 512], f32, tag="ps", name="psH1e")
                    nc.tensor.matmul(
                        ps[:], eWl0[0:64, 0, mc * 120:(mc + 1) * 120],
                        Ge0[:, nbs], start=True, stop=False)
                    nc.tensor.matmul(
                        ps[:], eWl0[:, 1, mc * 120:(mc + 1) * 120],
                        Ge1[:, nbs], start=False, stop=False)
                    nc.tensor.matmul(
                        ps[:], eWl0[0:108, 2, mc * 120:(mc + 1) * 120],
                        Ge2[:, nbs], start=False, stop=True)
                    nc.scalar.activation(H1e[:, mc, nbs], ps[:], Relu)
                    if mc < 3:
                        ps2 = psC.tile([100, 512], f32, tag="ps", name="psH1p")
                        nc.tensor.matmul(
                            ps2[:], pWl0[:, 0, mc * 100:(mc + 1) * 100],
                            Gp0[:, nbs], start=True, stop=False)
                        nc.tensor.matmul(
                            ps2[:], pWl0[0:32, 1, mc * 100:(mc + 1) * 100],
                            Gp1[:, nbs], start=False, stop=True)
                        nc.scalar.activation(H1p[:, mc, nbs], ps2[:], Relu)

            # logits+softmax interleaved with Z
            Z = bigp.tile([128, 8, 300], bf16, tag="Z")
            for m in range(8):
                ps = psC.tile([128, K0], f32, tag="ps", name="psL")
                for kc in range(3):
                    nc.tensor.matmul(
                        ps[:], H1p[:, kc, m * 128:(m + 1) * 128], pWo0[:, kc, :],
                        start=(kc == 0), stop=(kc == 2),
                    )
                psz = psC.tile([128, 300], f32, tag="ps", name="psZ")
                for kc in range(5):
                    nc.tensor.matmul(
                        psz[:], H1e[:, kc, m * 128:(m + 1) * 128], eWo0[:, kc, :],
                        start=(kc == 0), stop=(kc == 4),
                    )
                nc.vector.tensor_copy(Z[:, m, :], psz[:])
                s_sb = tmp.tile([128, K0], bf16, tag="s0")
                _softmax_rowmajor(nc, tmp, ps, s_sb, K0)
                nc.vector.tensor_copy(
                    S_bd[0:64, m, m * 20:m * 20 + 10], s_sb[0:64, :])
                nc.vector.tensor_copy(
                    S_bd[64:128, m, m * 20 + 10:m * 20 + 20], s_sb[64:128, :])

            # ---- level-0 pooling ----
            # X1T[300, 160] = Z^T @ S_bd
            X1T = bigp.tile([100, 3, 160], bf16, tag="X1T")
            for mc in range(3):
                ps = psC.tile([100, 160], f32, tag="ps", name="psX1T")
                for k in range(8):
                    nc.tensor.matmul(
                        ps[:], Z[:, k, mc * 100:(mc + 1) * 100], S_bd[:, k, :],
                        start=(k == 0), stop=(k == 7),
                    )
                nc.vector.tensor_copy(X1T[:, mc, :], ps[:])

            # T_bd = A0_bd @ S_bd ; T2_bd = A0_bd^T @ S_bd  (block diag)
            T_bd = bigp.tile([128, 8, 160], bf16, tag="T_bd")
            T2_bd = bigp.tile([128, 8, 160], bf16, tag="T2_bd")
            nc.any.memzero(T_bd[:])
            nc.any.memzero(T2_bd[:])
            for c in range(8):
                psT = psC.tile([128, 20], f32, tag="ps", name="psT")
                nc.tensor.matmul(psT[:], adiagT[:, c, :],
                                 S_bd[:, c, c * 20:c * 20 + 20],
                                 start=True, stop=True)
                nc.vector.tensor_copy(T_bd[:, c, c * 20:c * 20 + 20], psT[:])
                psT2 = psC.tile([128, 20], f32, tag="ps", name="psT2")
                nc.tensor.matmul(psT2[:], adiag[:, c, :],
                                 S_bd[:, c, c * 20:c * 20 + 20],
                                 start=True, stop=True)
                nc.vector.tensor_copy(T2_bd[:, c, c * 20:c * 20 + 20], psT2[:])

            # A1_bd = S_bd^T @ T_bd ; A1T_bd = S_bd^T @ T2_bd   [160, 160]
            A1bd = bigp.tile([80, 2, 160], bf16, tag="A1bd")
            A1Tbd = bigp.tile([80, 2, 160], bf16, tag="A1Tbd")
            for mc in range(2):
                ps1 = psC.tile([80, 160], f32, tag="ps", name="psA1")
                ps2 = psC.tile([80, 160], f32, tag="ps", name="psA1T")
                for k in range(8):
                    nc.tensor.matmul(
                        ps1[:], S_bd[:, k, mc * 80:(mc + 1) * 80], T_bd[:, k, :],
                        start=(k == 0), stop=(k == 7))
                for k in range(8):
                    nc.tensor.matmul(
                        ps2[:], S_bd[:, k, mc * 80:(mc + 1) * 80], T2_bd[:, k, :],
                        start=(k == 0), stop=(k == 7))
                nc.vector.tensor_copy(A1bd[:, mc, :], ps1[:])
                nc.vector.tensor_copy(A1Tbd[:, mc, :], ps2[:])

            # ---- level 1 ----
            # Y1p [160, 150], Y1e [160, 300] row-major
            Y1p = bigp.tile([80, 2, 150], bf16, tag="Y1p")
            Y1e = bigp.tile([80, 2, 300], bf16, tag="Y1e")
            for mi in range(2):
                psp = psC.tile([80, 150], f32, tag="ps", name="psY1p")
                pse = psC.tile([80, 300], f32, tag="ps", name="psY1e")
                for kc in range(3):
                    nc.tensor.matmul(
                        psp[:], X1T[:, kc, mi * 80:(mi + 1) * 80], pWh1[:, kc, :],
                        start=(kc == 0), stop=(kc == 2))
                for kc in range(3):
                    nc.tensor.matmul(
                        pse[:], X1T[:, kc, mi * 80:(mi + 1) * 80], eWh1[:, kc, :],
                        start=(kc == 0), stop=(kc == 2))
                nc.vector.tensor_copy(Y1p[:, mi, :], psp[:])
                nc.vector.tensor_copy(Y1e[:, mi, :], pse[:])

            # M1pt [150, 160] = Y1p^T @ A1T_bd, relu -> G1p [75, 2, 160]
            G1p = bigp.tile([75, 2, 160], bf16, tag="G1p")
            for mf in range(2):
                ps = psC.tile([75, 160], f32, tag="ps", name="psM1p")
                for kc in range(2):
                    nc.tensor.matmul(
                        ps[:], Y1p[:, kc, mf * 75:(mf + 1) * 75], A1Tbd[:, kc, :],
                        start=(kc == 0), stop=(kc == 1))
                nc.scalar.activation(G1p[:, mf, :], ps[:], Relu)

            G1e = bigp.tile([100, 3, 160], bf16, tag="G1e")
            for mf in range(3):
                ps = psC.tile([100, 160], f32, tag="ps", name="psM1e")
                for kc in range(2):
                    nc.tensor.matmul(
                        ps[:], Y1e[:, kc, mf * 100:(mf + 1) * 100], A1Tbd[:, kc, :],
                        start=(kc == 0), stop=(kc == 1))
                nc.scalar.activation(G1e[:, mf, :], ps[:], Relu)

            # pool chain level 1
            H1p1 = bigp.tile([100, 3, 160], bf16, tag="H1p1")
            for mc in range(3):
                ps = psC.tile([100, 160], f32, tag="ps", name="psH1p1")
                for kc in range(2):
                    nc.tensor.matmul(
                        ps[:], pWl1[:, kc, mc * 100:(mc + 1) * 100], G1p[:, kc, :],
                        start=(kc == 0), stop=(kc == 1))
                nc.scalar.activation(H1p1[:, mc, :], ps[:], Relu)

            H1e1 = bigp.tile([120, 5, 160], bf16, tag="H1e1")
            for mc in range(5):
                ps = psC.tile([120, 160], f32, tag="ps", name="psH1e1")
                for kc in range(3):
                    nc.tensor.matmul(
                        ps[:], eWl1[:, kc, mc * 120:(mc + 1) * 120], G1e[:, kc, :],
                        start=(kc == 0), stop=(kc == 2))
                nc.scalar.activation(H1e1[:, mc, :], ps[:], Relu)

            S1_bd = bigp.tile([80, 2, 64], bf16, tag="S1_bd")
            s1mask = load(d_s1mask, [80, 2, 64])
            for mi in range(2):
                ps = psC.tile([80, K1], f32, tag="ps", name="psL1")
                for kc in range(3):
                    nc.tensor.matmul(
                        ps[:], H1p1[:, kc, mi * 80:(mi + 1) * 80], pWo1[:, kc, :],
                        start=(kc == 0), stop=(kc == 2))
                s_sb = tmp.tile([80, K1], bf16, tag="s1")
                _softmax_rowmajor(nc, tmp, ps, s_sb, K1)
                # block-diag scatter: replicate the [80,4] softmax 16x along
                # free dim and mask to the owning graph's 4 columns
                nc.vector.tensor_tensor(
                    S1_bd[:, mi, :].rearrange("p (b j) -> p b j", j=K1),
                    s_sb[:, None, :].to_broadcast((80, GPC, K1)),
                    s1mask[:, mi, :].rearrange("p (b j) -> p b j", j=K1),
                    mybir.AluOpType.mult)

            Z1 = bigp.tile([80, 2, 300], bf16, tag="Z1")
            for mi in range(2):
                ps = psC.tile([80, 300], f32, tag="ps", name="psZ1")
                for kc in range(5):
                    nc.tensor.matmul(
                        ps[:], H1e1[:, kc, mi * 80:(mi + 1) * 80], eWo1[:, kc, :],
                        start=(kc == 0), stop=(kc == 4))
                nc.vector.tensor_copy(Z1[:, mi, :], ps[:])

            # pooling level 1
            X2T = bigp.tile([100, 3, 64], bf16, tag="X2T")
            for mc in range(3):
                ps = psC.tile([100, 64], f32, tag="ps", name="psX2T")
                for kc in range(2):
                    nc.tensor.matmul(
                        ps[:], Z1[:, kc, mc * 100:(mc + 1) * 100], S1_bd[:, kc, :],
                        start=(kc == 0), stop=(kc == 1))
                nc.vector.tensor_copy(X2T[:, mc, :], ps[:])

            # T3 = A1_bd^T @ S1_bd ; A2T_bd = S1_bd^T @ T3   [64, 64]
            T3 = bigp.tile([80, 2, 64], bf16, tag="T3")
            for mi in range(2):
                ps = psC.tile([80, 64], f32, tag="ps", name="psT3")
                for kc in range(2):
                    nc.tensor.matmul(
                        ps[:], A1bd[:, kc, mi * 80:(mi + 1) * 80], S1_bd[:, kc, :],
                        start=(kc == 0), stop=(kc == 1))
                nc.vector.tensor_copy(T3[:, mi, :], ps[:])
            A2Tbd = bigp.tile([64, 64], bf16, tag="A2Tbd")
            psA2 = psC.tile([64, 64], f32, tag="ps", name="psA2T")
            for kc in range(2):
                nc.tensor.matmul(
                    psA2[:], S1_bd[:, kc, :], T3[:, kc, :],
                    start=(kc == 0), stop=(kc == 1))
            nc.vector.tensor_copy(A2Tbd[:], psA2[:])

            # ---- level 2 (emb only; S2 == 1) ----
            Y2 = bigp.tile([64, 300], bf16, tag="Y2")
            psY2 = psC.tile([64, 300], f32, tag="ps", name="psY2")
            for kc in range(3):
                nc.tensor.matmul(
                    psY2[:], X2T[:, kc, 0:64], eWh2[:, kc, :],
                    start=(kc == 0), stop=(kc == 2))
            nc.vector.tensor_copy(Y2[:], psY2[:])

            G2 = bigp.tile([100, 3, 64], bf16, tag="G2")
            for mf in range(3):
                ps = psC.tile([100, 64], f32, tag="ps", name="psM2")
                nc.tensor.matmul(
                    ps[:], Y2[:, mf * 100:(mf + 1) * 100], A2Tbd[:],
                    start=True, stop=True)
                nc.scalar.activation(G2[:, mf, :], ps[:], Relu)

            H2 = bigp.tile([120, 5, 64], bf16, tag="H2")
            for mc in range(5):
                ps = psC.tile([120, 64], f32, tag="ps", name="psH2")
                for kc in range(3):
                    nc.tensor.matmul(
                        ps[:], eWl2[:, kc, mc * 120:(mc + 1) * 120], G2[:, kc, :],
                        start=(kc == 0), stop=(kc == 2))
                nc.scalar.activation(H2[:, mc, :], ps[:], Relu)

            Z2 = bigp.tile([64, 300], bf16, tag="Z2")
            psZ2 = psC.tile([64, 300], f32, tag="ps", name="psZ2")
            for kc in range(5):
                nc.tensor.matmul(
                    psZ2[:], H2[:, kc, 0:64], eWo2[:, kc, :],
                    start=(kc == 0), stop=(kc == 4))
            nc.vector.tensor_copy(Z2[:], psZ2[:])

            # X3T [300, 16] = Z2^T @ ones_bd
            X3T = bigp.tile([100, 3, GPC], bf16, tag="X3T")
            for mf in range(3):
                ps = psC.tile([100, GPC], f32, tag="ps", name="psX3T")
                nc.tensor.matmul(
                    ps[:], Z2[:, mf * 100:(mf + 1) * 100], ones16[:],
                    start=True, stop=True)
                nc.vector.tensor_copy(X3T[:, mf, :], ps[:])

            # ---- head ----
            hT = bigp.tile([120, 5, GPC], bf16, tag="hT")
            for mc in range(5):
                ps = psC.tile([120, GPC], f32, tag="ps", name="psh")
                for kc in range(3):
                    nc.tensor.matmul(
                        ps[:], lW1[:, kc, mc * 120:(mc + 1) * 120], X3T[:, kc, :],
                        start=(kc == 0), stop=(kc == 2))
                nc.scalar.activation(hT[:, mc, :], ps[:], Relu,
                                     bias=lb1[:, mc:mc + 1])

            psO = psC.tile([128, GPC], f32, tag="ps", name="psO")
            for kc in range(5):
                nc.tensor.matmul(
                    psO[:], lW2[:, kc, :], hT[:, kc, :],
                    start=(kc == 0), stop=(kc == 4))
            outT = tmp.tile([128, GPC], f32, tag="outT")
            nc.vector.tensor_scalar_add(outT[:], psO[:], lb2[:])
            nc.sync.dma_start(d_out[:], outT[:])

    _split_excess_waits(nc)
    return nc


def _host_prep(inputs):
    """Build per-core in_maps from the full inputs."""
    ONE = np.uint8(0x38)  # 1.0 in float8_e4m3

    x = np.asarray(inputs["x"], np.float32)
    ei = np.asarray(inputs["edge_index"]).astype(np.int64)

    # full A^T in bf16 bit pattern: AT[j, i] = A[i, j]
    ATu = np.zeros((N_NODES, N_NODES), np.uint8)
    ATu[ei[1], ei[0]] = ONE

    xT = np.ascontiguousarray(
        x.T.reshape(3, 100, 8, 1024).transpose(2, 1, 0, 3)).astype(BF)

    def chunkw(w, p, c):
        w = np.asarray(w, np.float32)
        return np.ascontiguousarray(
            w.reshape(c, p, -1).transpose(1, 0, 2)).astype(BF)

    def padchunk(w, rowchunks, c, m):
        w = np.asarray(w, np.float32)
        out = np.zeros((128, c, m), np.float32)
        for ci, (a, b) in enumerate(rowchunks):
            out[0:b - a, ci, :] = w[a:b, :]
        return out.astype(BF)

    wcat0 = np.zeros((300, 492), np.float32)
    wcat0[:, 0:150] = np.asarray(inputs["pWh0"], np.float32)
    wcat0[:, 192:492] = np.asarray(inputs["eWh0"], np.float32)

    ones16 = np.zeros((64, GPC), BF)
    for b in range(GPC):
        ones16[b * 4:(b + 1) * 4, b] = 1
    s1mask = np.zeros((80, 2, 64), BF)
    for mi in range(2):
        for p in range(80):
            gb = (80 * mi + p) // K1NODES
            s1mask[p, mi, gb * 4:(gb + 1) * 4] = 1
    lb1 = np.ascontiguousarray(
        np.asarray(inputs["lb1"], np.float32).reshape(5, 120).T)
    lb2 = np.asarray(inputs["lb2"], np.float32).reshape(128, 1)

    shared = {
        "xT": xT,
        "wcat0": chunkw(wcat0, 100, 3),
        "pWl0": padchunk(inputs["pWl0"], [(0, 128), (128, 150)], 2, 300),
        "pWo0": chunkw(inputs["pWo0"], 100, 3),
        "eWl0": padchunk(inputs["eWl0"], [(0, 64), (64, 192), (192, 300)], 3, 600),
        "eWo0": chunkw(inputs["eWo0"], 120, 5),
        "pWh1": chunkw(inputs["pWh1"], 100, 3),
        "pWl1": chunkw(inputs["pWl1"], 75, 2),
        "pWo1": chunkw(inputs["pWo1"], 100, 3),
        "eWh1": chunkw(inputs["eWh1"], 100, 3),
        "eWl1": chunkw(inputs["eWl1"], 100, 3),
        "eWo1": chunkw(inputs["eWo1"], 120, 5),
        "eWh2": chunkw(inputs["eWh2"], 100, 3),
        "eWl2": chunkw(inputs["eWl2"], 100, 3),
        "eWo2": chunkw(inputs["eWo2"], 120, 5),
        "lW1": chunkw(inputs["lW1"], 100, 3),
        "lW2": chunkw(inputs["lW2"], 120, 5),
        "lb1": lb1,
        "lb2": lb2,
        "ones16": ones16,
        "s1mask": s1mask,
    }

    in_maps = []
    for d in range(N_CORES):
        r0 = d * R
        slab = ATu[:, r0:r0 + R]  # [8192, 1024]
        at = np.ascontiguousarray(
            slab.reshape(32, 2, 128, 1024).transpose(0, 2, 1, 3)).view(F8)

        adiag = np.zeros((128, 8, 128), np.uint8)
        adiagT = np.zeros((128, 8, 128), np.uint8)
        for c in range(8):
            # full 128x128 slab block, then mask to per-graph 64x64 diag
            blkT = slab[r0 + 128 * c: r0 + 128 * (c + 1),
                        128 * c: 128 * (c + 1)]  # blkT[q, p] = A[rows p, cols q]
            blk = blkT.T
            for h in range(2):
                s = slice(64 * h, 64 * (h + 1))
                adiag[s, c, s] = blk[s, s]
                adiagT[s, c, s] = blkT[s, s]
        m = dict(shared)
        m["at"] = at
        m["adiag"] = adiag.view(F8).astype(BF)
        m["adiagT"] = adiagT.view(F8).astype(BF)
        in_maps.append(m)
    return in_maps


def _run(inputs, trace=False, trace_kwargs=None):
    try:
        import concourse.bass as bass  # noqa: F401
    except ImportError:
        import sys
        sys.path.insert(0, "/opt/trn_rl_repo")
    from concourse.bass_utils import run_bass_kernel_spmd

    if "prog" not in _prog_cache:
        _prog_cache["prog"] = _build_program()
    nc = _prog_cache["prog"]

    in_maps = _host_prep(inputs)
    res = run_bass_kernel_spmd(
        nc, in_maps, core_ids=list(range(N_CORES)), trace=trace,
        **(trace_kwargs or {}),
    )
    out = np.empty((B, 128), np.float32)
    for d in range(N_CORES):
        out[d * GPC:(d + 1) * GPC, :] = res.results[d]["out"].T
    return out, res


def kernel(**inputs):
    out, _ = _run(inputs, trace=False)
    return out



# revision 9
# speedup vs baseline: 1.2978x; 1.2978x over previous
"""DiffPool GNN encoder on 8 Trainium2 NeuronCores.

Data-parallel over graphs: core d owns graphs [16d, 16d+16) = node rows
[1024d, 1024d+1024). Host builds each core's dense A^T slab (bf16 0/1),
the per-graph block-diagonal 64x64 A blocks, x^T, and pre-chunked bf16
weights. The device kernel computes, per core:

  level 0:  MaggT = x^T @ AT_slab          (feature-major aggregation;
            A @ (x@W) == (A@x) @ W so raw x (fp8) is aggregated once)
            G = relu([pWh0|eWh0]^T @ MaggT)
            pool chain -> softmax S0 -> block-diag S_bd [1024,160]
            emb chain  -> Z [1024,300] row-major
            X1T = Z^T @ S_bd, A1 = S^T A S (+ transposed variant), both
            via block-diag matmuls
  level 1:  same, 16 graphs x 10 nodes
  level 2:  emb only (pool softmax over k=1 is identically 1), X3 = per
            graph column sum of Z2
  head:     out^T = lW2^T @ relu(lW1^T @ X3T + lb1) + lb2   [128, 16]

Host gathers the 8 [128,16] outputs into the [128,128] result.
"""

import numpy as np
import ml_dtypes

BF = ml_dtypes.bfloat16
F8 = ml_dtypes.float8_e4m3fn
N_CORES = 8
N_NODES = 8192
B = 128
GPC = 16          # graphs per core
R = 1024          # rows per core
N0 = 64           # nodes per graph at level 0
D = 300
K0, K1 = 10, 4
K1NODES = 10  # nodes per graph at level 1

_prog_cache = {}


def _patch_tile_drain():
    """This container's walrus rejects >2 sync waits on one instruction;
    split the Tile tail-drain waits across several drains."""
    import concourse.tile as tile_mod
    from concourse.vector_clock import ScopedClock, VectorClock

    if getattr(tile_mod.TileContext, "_drain_patched", False):
        return

    def _patched(self, tick_clock, wait_clock):
        gc = tick_clock.global_clock
        n = len(gc)
        for start in range(0, n, 2):
            partial = VectorClock(
                [gc[p] if start <= p < start + 2 else 0 for p in range(n)]
            )
            di = self.nc.sync.drain()
            wait_clock.add_sem_waits(di.ins, ScopedClock({None: partial}))
        self.nc.all_engine_barrier()
        assert self.sems is not None
        popped = self.nc._tile_sem_poison_stack.pop()
        assert popped is self._sem_poison
        self.nc.clear_and_free_semaphores(list(self.sems.allocated().values()))
        self.nc.all_engine_barrier()

    tile_mod.TileContext._drain_and_barrier = _patched
    tile_mod.TileContext._drain_patched = True


def _split_excess_waits(nc, max_waits=1):
    """walrus here rejects instructions with >2 sync waits. Move excess waits
    onto injected same-engine nops placed immediately before the instruction
    (engine queues execute in order, so this preserves semantics)."""
    import concourse.mybir as mybir

    blocks = nc.m.functions[0].blocks
    for b in blocks:
        idx = 0
        while idx < len(b.instructions):
            inst = b.instructions[idx]
            si = inst.sync_info
            lim = max_waits
            if si is None or not si.on_wait or len(si.on_wait) <= lim:
                idx += 1
                continue
            waits = list(si.on_wait)
            keep = waits[-lim:]
            rest = waits[:-lim]
            inst.sync_info = mybir.SyncInfo(
                on_wait=keep, on_update=list(si.on_update or []))
            nops = []
            for c0 in range(0, len(rest)):
                n = nc.engines[inst.engine].nop(nofuse=True)
                ni = n.ins
                ni.sync_info = mybir.SyncInfo(
                    on_wait=[rest[c0]], on_update=[])
                # remove from wherever the builder appended it
                for b2 in blocks:
                    for j in range(len(b2.instructions) - 1, -1, -1):
                        if b2.instructions[j] is ni:
                            b2.instructions.pop(j)
                            break
                nops.append(ni)
            for n_off, ni in enumerate(nops):
                b.instructions.insert(idx + n_off, ni)
            idx += len(nops) + 1


def _softmax_rowmajor(nc, pool, psum_l, out_sb, k):
    """Row-major softmax over free dim k. psum_l: [p, k] f32 logits;
    out_sb: [p, k] bf16 destination."""
    import concourse.mybir as mybir

    p = psum_l.shape[0]
    mx = pool.tile([p, 1], mybir.dt.float32, tag="smax_mx")
    sm = pool.tile([p, 1], mybir.dt.float32, tag="smax_sum")
    rc = pool.tile([p, 1], mybir.dt.float32, tag="smax_rcp")
    ex = pool.tile([p, k], mybir.dt.float32, tag="smax_exp")
    nc.vector.reduce_max(mx[:], psum_l[:], axis=mybir.AxisListType.X, negate=True)
    nc.scalar.activation(
        ex[:], psum_l[:], mybir.ActivationFunctionType.Exp,
        bias=mx[:], scale=1.0, accum_out=sm[:],
    )
    nc.vector.reciprocal(rc[:], sm[:])
    nc.vector.tensor_scalar_mul(out_sb[:], ex[:], rc[:])


def _build_program():
    import concourse.bass as bass
    import concourse.mybir as mybir
    import concourse.tile as tile

    _patch_tile_drain()
    f32 = mybir.dt.float32
    bf16 = mybir.dt.bfloat16
    fp8 = mybir.dt.float8e4

    nc = bass.Bass()

    # ---- DRAM inputs (per-core shards handed via in_maps) ----
    d_x8 = nc.dram_tensor("x8", [8, 128, 8, 304], fp8, kind="ExternalInput")
    d_at = nc.dram_tensor("at", [32, 128, 2, 1024], fp8, kind="ExternalInput")
    d_adiag = nc.dram_tensor("adiag", [128, 8, 128], bf16, kind="ExternalInput")
    d_adiagT = nc.dram_tensor("adiagT", [128, 8, 128], bf16, kind="ExternalInput")
    d_wcat0 = nc.dram_tensor("wcat0", [100, 3, 492], bf16, kind="ExternalInput")
    d_pWl0 = nc.dram_tensor("pWl0", [128, 2, 300], bf16, kind="ExternalInput")
    d_pWo0 = nc.dram_tensor("pWo0", [100, 3, K0], bf16, kind="ExternalInput")
    d_eWl0 = nc.dram_tensor("eWl0", [128, 3, 600], bf16, kind="ExternalInput")
    d_eWo0 = nc.dram_tensor("eWo0", [120, 5, 300], bf16, kind="ExternalInput")
    d_pWh1 = nc.dram_tensor("pWh1", [100, 3, 150], bf16, kind="ExternalInput")
    d_pWl1 = nc.dram_tensor("pWl1", [75, 2, 300], bf16, kind="ExternalInput")
    d_pWo1 = nc.dram_tensor("pWo1", [100, 3, K1], bf16, kind="ExternalInput")
    d_eWh1 = nc.dram_tensor("eWh1", [100, 3, 300], bf16, kind="ExternalInput")
    d_eWl1 = nc.dram_tensor("eWl1", [100, 3, 600], bf16, kind="ExternalInput")
    d_eWo1 = nc.dram_tensor("eWo1", [120, 5, 300], bf16, kind="ExternalInput")
    d_eWh2 = nc.dram_tensor("eWh2", [100, 3, 300], bf16, kind="ExternalInput")
    d_eWl2 = nc.dram_tensor("eWl2", [100, 3, 600], bf16, kind="ExternalInput")
    d_eWo2 = nc.dram_tensor("eWo2", [120, 5, 300], bf16, kind="ExternalInput")
    d_lW1 = nc.dram_tensor("lW1", [100, 3, 600], bf16, kind="ExternalInput")
    d_lW2 = nc.dram_tensor("lW2", [120, 5, 128], bf16, kind="ExternalInput")
    d_lb1 = nc.dram_tensor("lb1", [120, 5], f32, kind="ExternalInput")
    d_lb2 = nc.dram_tensor("lb2", [128, 1], f32, kind="ExternalInput")
    d_ones = nc.dram_tensor("ones16", [64, GPC], bf16, kind="ExternalInput")
    d_s1mask = nc.dram_tensor("s1mask", [80, 2, 64], bf16, kind="ExternalInput")
    d_out = nc.dram_tensor("out", [128, GPC], f32, kind="ExternalOutput")

    with tile.TileContext(nc) as tc:
        with (
            tc.tile_pool(name="wpool", bufs=1) as wp,      # resident weights
            tc.tile_pool(name="big", bufs=1) as bigp,      # resident activations
            tc.tile_pool(name="atp", bufs=6) as atp,       # streamed AT tiles
            tc.tile_pool(name="tmp", bufs=4) as tmp,       # small temporaries
            tc.tile_pool(name="ps", bufs=8, space="PSUM") as psC,
        ):
            def load(dram, shape, eng=None):
                t = wp.tile(shape, dram.dtype, tag=dram.name)
                (eng or nc.scalar).dma_start(t[:], dram[:])
                return t

            # first at chunks + x8 chunks 0-1 lead the DMA order so the
            # aggregation matmuls can start immediately; remaining x8 chunks
            # are issued lazily inside the kk loop, weights go on scalar.
            # DMA-capable queues: sync (SP), scalar (Activation), gpsimd.
            x8 = wp.tile([128, 64, 304], fp8, tag="x8")
            dma_engs = [nc.sync, nc.gpsimd]

            def load_x8(nq):
                dma_engs[nq % 2].dma_start(
                    x8[:, nq * 8:(nq + 1) * 8, :], d_x8[nq])

            at_tiles = {}
            for c in (0, 1):
                at_t = atp.tile([128, 2, 1024], fp8, tag="at")
                dma_engs[c % 2].dma_start(at_t[:], d_at[c])
                at_tiles[c] = at_t
            load_x8(0)
            load_x8(1)
            wcat0 = load(d_wcat0, [100, 3, 492])
            pWl0 = load(d_pWl0, [128, 2, 300])
            pWo0 = load(d_pWo0, [100, 3, K0])
            eWl0 = load(d_eWl0, [128, 3, 600])
            eWo0 = load(d_eWo0, [120, 5, 300])
            adiag = load(d_adiag, [128, 8, 128])
            adiagT = load(d_adiagT, [128, 8, 128])
            pWh1 = load(d_pWh1, [100, 3, 150])
            pWl1 = load(d_pWl1, [75, 2, 300])
            pWo1 = load(d_pWo1, [100, 3, K1])
            eWh1 = load(d_eWh1, [100, 3, 300])
            eWl1 = load(d_eWl1, [100, 3, 600])
            eWo1 = load(d_eWo1, [120, 5, 300])
            eWh2 = load(d_eWh2, [100, 3, 300])
            eWl2 = load(d_eWl2, [100, 3, 600])
            eWo2 = load(d_eWo2, [120, 5, 300])
            lW1 = load(d_lW1, [100, 3, 600])
            lW2 = load(d_lW2, [120, 5, 128])
            lb1 = load(d_lb1, [120, 5])
            lb2 = load(d_lb2, [128, 1])
            ones16 = load(d_ones, [64, GPC])

            Relu = mybir.ActivationFunctionType.Relu

            # ---- stage B: MaggT = x^T @ AT  (feature-major aggregation) ----
            # A @ (x @ W) == (A @ x) @ W: aggregate the raw 300 features once,
            # then apply [pWh0|eWh0] to the [300, 1024] result.
            pss = [[psC.tile([100, 512], f32, tag="ps", name=f"psB_{nb}_{mi}")
                    for mi in range(3)] for nb in range(2)]
            for kk in range(0, 64, 2):
                c = kk // 2
                if kk % 8 == 0 and kk // 8 + 2 <= 7:
                    load_x8(kk // 8 + 2)
                if c in at_tiles:
                    at_t = at_tiles.pop(c)
                else:
                    at_t = atp.tile([128, 2, 1024], fp8, tag="at")
                    dma_engs[c % 2].dma_start(at_t[:], d_at[c])
                for nb in range(2):
                    for mi in range(3):
                        nc.tensor.matmul(
                            pss[nb][mi][:],
                            x8[:, kk:kk + 2, mi * 100:(mi + 1) * 100],
                            at_t[:, :, nb * 512:(nb + 1) * 512],
                            start=(kk == 0), stop=(kk == 62),
                            perf_mode=mybir.MatmulPerfMode.DoubleRow,
                        )
            MaggT = bigp.tile([100, 3, 1024], bf16, tag="MaggT")
            for nb in range(2):
                for mi in range(3):
                    nc.vector.tensor_copy(
                        MaggT[:, mi, nb * 512:(nb + 1) * 512], pss[nb][mi][:])

            # ---- weight-apply: G = relu(wcat0^T @ MaggT), feature-major ----
            # wcat0 column layout: pool [0:150) pad [150:192) emb [192:492)
            Gp0 = bigp.tile([128, 1024], bf16, tag="Gp0")
            Gp1 = bigp.tile([32, 1024], bf16, tag="Gp1")
            Ge0 = bigp.tile([64, 1024], bf16, tag="Ge0")
            Ge1 = bigp.tile([128, 1024], bf16, tag="Ge1")
            Ge2 = bigp.tile([108, 1024], bf16, tag="Ge2")
            gchunks = [(0, Gp0), (128, Gp1), (192, Ge0), (256, Ge1), (384, Ge2)]
            for nb in range(2):
                nbs = slice(nb * 512, (nb + 1) * 512)
                for goff, gdst in gchunks:
                    gsz = gdst.shape[0]
                    ps = psC.tile([gsz, 512], f32, tag="ps", name="psG")
                    for kc in range(3):
                        nc.tensor.matmul(
                            ps[:], wcat0[:, kc, goff:goff + gsz],
                            MaggT[:, kc, nbs],
                            start=(kc == 0), stop=(kc == 2),
                        )
                    nc.scalar.activation(gdst[:, nbs], ps[:], Relu)

            # ---- level-0 chains (pool/emb interleaved for PE density) ----
            H1p = bigp.tile([100, 3, 1024], bf16, tag="H1p")
            H1e = bigp.tile([120, 5, 1024], bf16, tag="H1e")
            S_bd = bigp.tile([128, 8, 160], bf16, tag="S_bd")
            nc.any.memzero(S_bd[:])
            for nb in range(2):
                nbs = slice(nb * 512, (nb + 1) * 512)
                for mc in range(5):
                    ps = psC.tile([120, 512], f32, tag="ps", name="psH1e")
                    nc.tensor.matmul(
                        ps[:], eWl0[0:64, 0, mc * 120:(mc + 1) * 120],
                        Ge0[:, nbs], start=True, stop=False)
                    nc.tensor.matmul(
                        ps[:], eWl0[:, 1, mc * 120:(mc + 1) * 120],
                        Ge1[:, nbs], start=False, stop=False)
                    nc.tensor.matmul(
                        ps[:], eWl0[0:108, 2, mc * 120:(mc + 1) * 120],
                        Ge2[:, nbs], start=False, stop=True)
                    nc.scalar.activation(H1e[:, mc, nbs], ps[:], Relu)
                    if mc < 3:
                        ps2 = psC.tile([100, 512], f32, tag="ps", name="psH1p")
                        nc.tensor.matmul(
                            ps2[:], pWl0[:, 0, mc * 100:(mc + 1) * 100],
                            Gp0[:, nbs], start=True, stop=False)
                        nc.tensor.matmul(
                            ps2[:], pWl0[0:32, 1, mc * 100:(mc + 1) * 100],
                            Gp1[:, nbs], start=False, stop=True)
                        nc.scalar.activation(H1p[:, mc, nbs], ps2[:], Relu)

            # logits+softmax interleaved with Z
            Z = bigp.tile([128, 8, 300], bf16, tag="Z")
            for m in range(8):
                ps = psC.tile([128, K0], f32, tag="ps", name="psL")
                for kc in range(3):
                    nc.tensor.matmul(
                        ps[:], H1p[:, kc, m * 128:(m + 1) * 128], pWo0[:, kc, :],
                        start=(kc == 0), stop=(kc == 2),
                    )
                psz = psC.tile([128, 300], f32, tag="ps", name="psZ")
                for kc in range(5):
                    nc.tensor.matmul(
                        psz[:], H1e[:, kc, m * 128:(m + 1) * 128], eWo0[:, kc, :],
                        start=(kc == 0), stop=(kc == 4),
                    )
                nc.vector.tensor_copy(Z[:, m, :], psz[:])
                s_sb = tmp.tile([128, K0], bf16, tag="s0")
                _softmax_rowmajor(nc, tmp, ps, s_sb, K0)
                nc.vector.tensor_copy(
                    S_bd[0:64, m, m * 20:m * 20 + 10], s_sb[0:64, :])
                nc.vector.tensor_copy(
                    S_bd[64:128, m, m * 20 + 10:m * 20 + 20], s_sb[64:128, :])

            # ---- level-0 pooling ----
            # X1T[300, 160] = Z^T @ S_bd
            X1T = bigp.tile([100, 3, 160], bf16, tag="X1T")
            for mc in range(3):
                ps = psC.tile([100, 160], f32, tag="ps", name="psX1T")
                for k in range(8):
                    nc.tensor.matmul(
                        ps[:], Z[:, k, mc * 100:(mc + 1) * 100], S_bd[:, k, :],
                        start=(k == 0), stop=(k == 7),
                    )
                nc.vector.tensor_copy(X1T[:, mc, :], ps[:])

            # T_bd = A0_bd @ S_bd ; T2_bd = A0_bd^T @ S_bd  (block diag)
            T_bd = bigp.tile([128, 8, 160], bf16, tag="T_bd")
            T2_bd = bigp.tile([128, 8, 160], bf16, tag="T2_bd")
            nc.any.memzero(T_bd[:])
            nc.any.memzero(T2_bd[:])
            for c in range(8):
                psT = psC.tile([128, 20], f32, tag="ps", name="psT")
                nc.tensor.matmul(psT[:], adiagT[:, c, :],
                                 S_bd[:, c, c * 20:c * 20 + 20],
                                 start=True, stop=True)
                nc.vector.tensor_copy(T_bd[:, c, c * 20:c * 20 + 20], psT[:])
                psT2 = psC.tile([128, 20], f32, tag="ps", name="psT2")
                nc.tensor.matmul(psT2[:], adiag[:, c, :],
                                 S_bd[:, c, c * 20:c * 20 + 20],
                                 start=True, stop=True)
                nc.vector.tensor_copy(T2_bd[:, c, c * 20:c * 20 + 20], psT2[:])

            # A1_bd = S_bd^T @ T_bd ; A1T_bd = S_bd^T @ T2_bd   [160, 160]
            A1bd = bigp.tile([80, 2, 160], bf16, tag="A1bd")
            A1Tbd = bigp.tile([80, 2, 160], bf16, tag="A1Tbd")
            for mc in range(2):
                ps1 = psC.tile([80, 160], f32, tag="ps", name="psA1")
                ps2 = psC.tile([80, 160], f32, tag="ps", name="psA1T")
                for k in range(8):
                    nc.tensor.matmul(
                        ps1[:], S_bd[:, k, mc * 80:(mc + 1) * 80], T_bd[:, k, :],
                        start=(k == 0), stop=(k == 7))
                for k in range(8):
                    nc.tensor.matmul(
                        ps2[:], S_bd[:, k, mc * 80:(mc + 1) * 80], T2_bd[:, k, :],
                        start=(k == 0), stop=(k == 7))
                nc.vector.tensor_copy(A1bd[:, mc, :], ps1[:])
                nc.vector.tensor_copy(A1Tbd[:, mc, :], ps2[:])

            # ---- level 1 ----
            # Y1p [160, 150], Y1e [160, 300] row-major
            Y1p = bigp.tile([80, 2, 150], bf16, tag="Y1p")
            Y1e = bigp.tile([80, 2, 300], bf16, tag="Y1e")
            for mi in range(2):
                psp = psC.tile([80, 150], f32, tag="ps", name="psY1p")
                pse = psC.tile([80, 300], f32, tag="ps", name="psY1e")
                for kc in range(3):
                    nc.tensor.matmul(
                        psp[:], X1T[:, kc, mi * 80:(mi + 1) * 80], pWh1[:, kc, :],
                        start=(kc == 0), stop=(kc == 2))
                for kc in range(3):
                    nc.tensor.matmul(
                        pse[:], X1T[:, kc, mi * 80:(mi + 1) * 80], eWh1[:, kc, :],
                        start=(kc == 0), stop=(kc == 2))
                nc.vector.tensor_copy(Y1p[:, mi, :], psp[:])
                nc.vector.tensor_copy(Y1e[:, mi, :], pse[:])

            # M1pt [150, 160] = Y1p^T @ A1T_bd, relu -> G1p [75, 2, 160]
            G1p = bigp.tile([75, 2, 160], bf16, tag="G1p")
            for mf in range(2):
                ps = psC.tile([75, 160], f32, tag="ps", name="psM1p")
                for kc in range(2):
                    nc.tensor.matmul(
                        ps[:], Y1p[:, kc, mf * 75:(mf + 1) * 75], A1Tbd[:, kc, :],
                        start=(kc == 0), stop=(kc == 1))
                nc.scalar.activation(G1p[:, mf, :], ps[:], Relu)

            G1e = bigp.tile([100, 3, 160], bf16, tag="G1e")
            for mf in range(3):
                ps = psC.tile([100, 160], f32, tag="ps", name="psM1e")
                for kc in range(2):
                    nc.tensor.matmul(
                        ps[:], Y1e[:, kc, mf * 100:(mf + 1) * 100], A1Tbd[:, kc, :],
                        start=(kc == 0), stop=(kc == 1))
                nc.scalar.activation(G1e[:, mf, :], ps[:], Relu)

            # pool chain level 1
            H1p1 = bigp.tile([100, 3, 160], bf16, tag="H1p1")
            for mc in range(3):
                ps = psC.tile([100, 160], f32, tag="ps", name="psH1p1")
                for kc in range(2):
                    nc.tensor.matmul(
                        ps[:], pWl1[:, kc, mc * 100:(mc + 1) * 100], G1p[:, kc, :],
                        start=(kc == 0), stop=(kc == 1))
                nc.scalar.activation(H1p1[:, mc, :], ps[:], Relu)

            H1e1 = bigp.tile([120, 5, 160], bf16, tag="H1e1")
            for mc in range(5):
                ps = psC.tile([120, 160], f32, tag="ps", name="psH1e1")
                for kc in range(3):
                    nc.tensor.matmul(
                        ps[:], eWl1[:, kc, mc * 120:(mc + 1) * 120], G1e[:, kc, :],
                        start=(kc == 0), stop=(kc == 2))
                nc.scalar.activation(H1e1[:, mc, :], ps[:], Relu)

            S1_bd = bigp.tile([80, 2, 64], bf16, tag="S1_bd")
            s1mask = load(d_s1mask, [80, 2, 64])
            for mi in range(2):
                ps = psC.tile([80, K1], f32, tag="ps", name="psL1")
                for kc in range(3):
                    nc.tensor.matmul(
                        ps[:], H1p1[:, kc, mi * 80:(mi + 1) * 80], pWo1[:, kc, :],
                        start=(kc == 0), stop=(kc == 2))
                s_sb = tmp.tile([80, K1], bf16, tag="s1")
                _softmax_rowmajor(nc, tmp, ps, s_sb, K1)
                # block-diag scatter: replicate the [80,4] softmax 16x along
                # free dim and mask to the owning graph's 4 columns
                nc.vector.tensor_tensor(
                    S1_bd[:, mi, :].rearrange("p (b j) -> p b j", j=K1),
                    s_sb[:, None, :].to_broadcast((80, GPC, K1)),
                    s1mask[:, mi, :].rearrange("p (b j) -> p b j", j=K1),
                    mybir.AluOpType.mult)

            Z1 = bigp.tile([80, 2, 300], bf16, tag="Z1")
            for mi in range(2):
                ps = psC.tile([80, 300], f32, tag="ps", name="psZ1")
                for kc in range(5):
                    nc.tensor.matmul(
                        ps[:], H1e1[:, kc, mi * 80:(mi + 1) * 80], eWo1[:, kc, :],
                        start=(kc == 0), stop=(kc == 4))
                nc.vector.tensor_copy(Z1[:, mi, :], ps[:])

            # pooling level 1
            X2T = bigp.tile([100, 3, 64], bf16, tag="X2T")
            for mc in range(3):
                ps = psC.tile([100, 64], f32, tag="ps", name="psX2T")
                for kc in range(2):
                    nc.tensor.matmul(
                        ps[:], Z1[:, kc, mc * 100:(mc + 1) * 100], S1_bd[:, kc, :],
                        start=(kc == 0), stop=(kc == 1))
                nc.vector.tensor_copy(X2T[:, mc, :], ps[:])

            # T3 = A1_bd^T @ S1_bd ; A2T_bd = S1_bd^T @ T3   [64, 64]
            T3 = bigp.tile([80, 2, 64], bf16, tag="T3")
            for mi in range(2):
                ps = psC.tile([80, 64], f32, tag="ps", name="psT3")
                for kc in range(2):
                    nc.tensor.matmul(
                        ps[:], A1bd[:, kc, mi * 80:(mi + 1) * 80], S1_bd[:, kc, :],
                        start=(kc == 0), stop=(kc == 1))
                nc.vector.tensor_copy(T3[:, mi, :], ps[:])
            A2Tbd = bigp.tile([64, 64], bf16, tag="A2Tbd")
            psA2 = psC.tile([64, 64], f32, tag="ps", name="psA2T")
            for kc in range(2):
                nc.tensor.matmul(
                    psA2[:], S1_bd[:, kc, :], T3[:, kc, :],
                    start=(kc == 0), stop=(kc == 1))
            nc.vector.tensor_copy(A2Tbd[:], psA2[:])

            # ---- level 2 (emb only; S2 == 1) ----
            Y2 = bigp.tile([64, 300], bf16, tag="Y2")
            psY2 = psC.tile([64, 300], f32, tag="ps", name="psY2")
            for kc in range(3):
                nc.tensor.matmul(
                    psY2[:], X2T[:, kc, 0:64], eWh2[:, kc, :],
                    start=(kc == 0), stop=(kc == 2))
            nc.vector.tensor_copy(Y2[:], psY2[:])

            G2 = bigp.tile([100, 3, 64], bf16, tag="G2")
            for mf in range(3):
                ps = psC.tile([100, 64], f32, tag="ps", name="psM2")
                nc.tensor.matmul(
                    ps[:], Y2[:, mf * 100:(mf + 1) * 100], A2Tbd[:],
                    start=True, stop=True)
                nc.scalar.activation(G2[:, mf, :], ps[:], Relu)

            H2 = bigp.tile([120, 5, 64], bf16, tag="H2")
            for mc in range(5):
                ps = psC.tile([120, 64], f32, tag="ps", name="psH2")
                for kc in range(3):
                    nc.tensor.matmul(
                        ps[:], eWl2[:, kc, mc * 120:(mc + 1) * 120], G2[:, kc, :],
                        start=(kc == 0), stop=(kc == 2))
                nc.scalar.activation(H2[:, mc, :], ps[:], Relu)

            Z2 = bigp.tile([64, 300], bf16, tag="Z2")
            psZ2 = psC.tile([64, 300], f32, tag="ps", name="psZ2")
            for kc in range(5):
                nc.tensor.matmul(
                    psZ2[:], H2[:, kc, 0:64], eWo2[:, kc, :],
                    start=(kc == 0), stop=(kc == 4))
            nc.vector.tensor_copy(Z2[:], psZ2[:])

            # X3T [300, 16] = Z2^T @ ones_bd
            X3T = bigp.tile([100, 3, GPC], bf16, tag="X3T")
            for mf in range(3):
                ps = psC.tile([100, GPC], f32, tag="ps", name="psX3T")
                nc.tensor.matmul(
                    ps[:], Z2[:, mf * 100:(mf + 1) * 100], ones16[:],
                    start=True, stop=True)
                nc.vector.tensor_copy(X3T[:, mf, :], ps[:])

            # ---- head ----
            hT = bigp.tile([120, 5, GPC], bf16, tag="hT")
            for mc in range(5):
                ps = psC.tile([120, GPC], f32, tag="ps", name="psh")
                for kc in range(3):
                    nc.tensor.matmul(
                        ps[:], lW1[:, kc, mc * 120:(mc + 1) * 120], X3T[:, kc, :],
                        start=(kc == 0), stop=(kc == 2))
                nc.scalar.activation(hT[:, mc, :], ps[:], Relu,
                                     bias=lb1[:, mc:mc + 1])

            psO = psC.tile([128, GPC], f32, tag="ps", name="psO")
            for kc in range(5):
                nc.tensor.matmul(
                    psO[:], lW2[:, kc, :], hT[:, kc, :],
                    start=(kc == 0), stop=(kc == 4))
            outT = tmp.tile([128, GPC], f32, tag="outT")
            nc.vector.tensor_scalar_add(outT[:], psO[:], lb2[:])
            nc.sync.dma_start(d_out[:], outT[:])

    _split_excess_waits(nc)
    return nc


def _host_prep(inputs):
    """Build per-core in_maps from the full inputs."""
    ONE = np.uint8(0x38)  # 1.0 in float8_e4m3

    x = np.asarray(inputs["x"], np.float32)
    ei = np.asarray(inputs["edge_index"]).astype(np.int64)

    # full A^T in bf16 bit pattern: AT[j, i] = A[i, j]
    ATu = np.zeros((N_NODES, N_NODES), np.uint8)
    ATu[ei[1], ei[0]] = ONE

    # x8[q][p, m, f] = x[128*(8q+m) + p, f], padded 300 -> 304 so the
    # DoubleRow Ko step (304 B) stays 16-byte aligned
    xp = np.zeros((N_NODES, 304), np.float32)
    xp[:, 0:300] = x
    x8 = np.ascontiguousarray(
        xp.reshape(8, 8, 128, 304).transpose(0, 2, 1, 3)).astype(F8)

    def chunkw(w, p, c):
        w = np.asarray(w, np.float32)
        return np.ascontiguousarray(
            w.reshape(c, p, -1).transpose(1, 0, 2)).astype(BF)

    def padchunk(w, rowchunks, c, m):
        w = np.asarray(w, np.float32)
        out = np.zeros((128, c, m), np.float32)
        for ci, (a, b) in enumerate(rowchunks):
            out[0:b - a, ci, :] = w[a:b, :]
        return out.astype(BF)

    wcat0 = np.zeros((300, 492), np.float32)
    wcat0[:, 0:150] = np.asarray(inputs["pWh0"], np.float32)
    wcat0[:, 192:492] = np.asarray(inputs["eWh0"], np.float32)

    ones16 = np.zeros((64, GPC), BF)
    for b in range(GPC):
        ones16[b * 4:(b + 1) * 4, b] = 1
    s1mask = np.zeros((80, 2, 64), BF)
    for mi in range(2):
        for p in range(80):
            gb = (80 * mi + p) // K1NODES
            s1mask[p, mi, gb * 4:(gb + 1) * 4] = 1
    lb1 = np.ascontiguousarray(
        np.asarray(inputs["lb1"], np.float32).reshape(5, 120).T)
    lb2 = np.asarray(inputs["lb2"], np.float32).reshape(128, 1)

    shared = {
        "x8": x8,
        "wcat0": chunkw(wcat0, 100, 3),
        "pWl0": padchunk(inputs["pWl0"], [(0, 128), (128, 150)], 2, 300),
        "pWo0": chunkw(inputs["pWo0"], 100, 3),
        "eWl0": padchunk(inputs["eWl0"], [(0, 64), (64, 192), (192, 300)], 3, 600),
        "eWo0": chunkw(inputs["eWo0"], 120, 5),
        "pWh1": chunkw(inputs["pWh1"], 100, 3),
        "pWl1": chunkw(inputs["pWl1"], 75, 2),
        "pWo1": chunkw(inputs["pWo1"], 100, 3),
        "eWh1": chunkw(inputs["eWh1"], 100, 3),
        "eWl1": chunkw(inputs["eWl1"], 100, 3),
        "eWo1": chunkw(inputs["eWo1"], 120, 5),
        "eWh2": chunkw(inputs["eWh2"], 100, 3),
        "eWl2": chunkw(inputs["eWl2"], 100, 3),
        "eWo2": chunkw(inputs["eWo2"], 120, 5),
        "lW1": chunkw(inputs["lW1"], 100, 3),
        "lW2": chunkw(inputs["lW2"], 120, 5),
        "lb1": lb1,
        "lb2": lb2,
        "ones16": ones16,
        "s1mask": s1mask,
    }

    in_maps = []
    for d in range(N_CORES):
        r0 = d * R
        slab = ATu[:, r0:r0 + R]  # [8192, 1024]
        at = np.ascontiguousarray(
            slab.reshape(32, 2, 128, 1024).transpose(0, 2, 1, 3)).view(F8)

        adiag = np.zeros((128, 8, 128), np.uint8)
        adiagT = np.zeros((128, 8, 128), np.uint8)
        for c in range(8):
            # full 128x128 slab block, then mask to per-graph 64x64 diag
            blkT = slab[r0 + 128 * c: r0 + 128 * (c + 1),
                        128 * c: 128 * (c + 1)]  # blkT[q, p] = A[rows p, cols q]
            blk = blkT.T
            for h in range(2):
                s = slice(64 * h, 64 * (h + 1))
                adiag[s, c, s] = blk[s, s]
                adiagT[s, c, s] = blkT[s, s]
        m = dict(shared)
        m["at"] = at
        m["adiag"] = adiag.view(F8).astype(BF)
        m["adiagT"] = adiagT.view(F8).astype(BF)
        in_maps.append(m)
    return in_maps


def _run(inputs, trace=False, trace_kwargs=None):
    try:
        import concourse.bass as bass  # noqa: F401
    except ImportError:
        import sys
        sys.path.insert(0, "/opt/trn_rl_repo")
    from concourse.bass_utils import run_bass_kernel_spmd

    if "prog" not in _prog_cache:
        _prog_cache["prog"] = _build_program()
    nc = _prog_cache["prog"]

    in_maps = _host_prep(inputs)
    res = run_bass_kernel_spmd(
        nc, in_maps, core_ids=list(range(N_CORES)), trace=trace,
        **(trace_kwargs or {}),
    )
    out = np.empty((B, 128), np.float32)
    for d in range(N_CORES):
        out[d * GPC:(d + 1) * GPC, :] = res.results[d]["out"].T
    return out, res


def kernel(**inputs):
    out, _ = _run(inputs, trace=False)
    return out



# revision 18
# speedup vs baseline: 1.3037x; 1.0045x over previous
"""DiffPool GNN encoder on 8 Trainium2 NeuronCores.

Data-parallel over graphs: core d owns graphs [16d, 16d+16) = node rows
[1024d, 1024d+1024). Host builds each core's dense A^T slab (bf16 0/1),
the per-graph block-diagonal 64x64 A blocks, x^T, and pre-chunked bf16
weights. The device kernel computes, per core:

  level 0:  MaggT = x^T @ AT_slab          (feature-major aggregation;
            A @ (x@W) == (A@x) @ W so raw x (fp8) is aggregated once)
            G = relu([pWh0|eWh0]^T @ MaggT)
            pool chain -> softmax S0 -> block-diag S_bd [1024,160]
            emb chain  -> Z [1024,300] row-major
            X1T = Z^T @ S_bd, A1 = S^T A S (+ transposed variant), both
            via block-diag matmuls
  level 1:  same, 16 graphs x 10 nodes
  level 2:  emb only (pool softmax over k=1 is identically 1), X3 = per
            graph column sum of Z2
  head:     out^T = lW2^T @ relu(lW1^T @ X3T + lb1) + lb2   [128, 16]

Host gathers the 8 [128,16] outputs into the [128,128] result.
"""

import numpy as np
import ml_dtypes

BF = ml_dtypes.bfloat16
F8 = ml_dtypes.float8_e4m3fn
N_CORES = 8
N_NODES = 8192
B = 128
GPC = 16          # graphs per core
R = 1024          # rows per core
N0 = 64           # nodes per graph at level 0
D = 300
K0, K1 = 10, 4
K1NODES = 10  # nodes per graph at level 1

_prog_cache = {}


def _patch_tile_drain():
    """This container's walrus rejects >2 sync waits on one instruction;
    split the Tile tail-drain waits across several drains."""
    import concourse.tile as tile_mod
    from concourse.vector_clock import ScopedClock, VectorClock

    if getattr(tile_mod.TileContext, "_drain_patched", False):
        return

    def _patched(self, tick_clock, wait_clock):
        gc = tick_clock.global_clock
        n = len(gc)
        for start in range(0, n, 2):
            partial = VectorClock(
                [gc[p] if start <= p < start + 2 else 0 for p in range(n)]
            )
            di = self.nc.sync.drain()
            wait_clock.add_sem_waits(di.ins, ScopedClock({None: partial}))
        self.nc.all_engine_barrier()
        assert self.sems is not None
        popped = self.nc._tile_sem_poison_stack.pop()
        assert popped is self._sem_poison
        self.nc.clear_and_free_semaphores(list(self.sems.allocated().values()))
        self.nc.all_engine_barrier()

    tile_mod.TileContext._drain_and_barrier = _patched
    tile_mod.TileContext._drain_patched = True


def _split_excess_waits(nc, max_waits=1):
    """walrus here rejects instructions with >2 sync waits. Move excess waits
    onto injected same-engine nops placed immediately before the instruction
    (engine queues execute in order, so this preserves semantics)."""
    import concourse.mybir as mybir

    blocks = nc.m.functions[0].blocks
    for b in blocks:
        idx = 0
        while idx < len(b.instructions):
            inst = b.instructions[idx]
            si = inst.sync_info
            lim = max_waits
            if si is None or not si.on_wait or len(si.on_wait) <= lim:
                idx += 1
                continue
            waits = list(si.on_wait)
            keep = waits[-lim:]
            rest = waits[:-lim]
            inst.sync_info = mybir.SyncInfo(
                on_wait=keep, on_update=list(si.on_update or []))
            nops = []
            for c0 in range(0, len(rest)):
                n = nc.engines[inst.engine].nop(nofuse=True)
                ni = n.ins
                ni.sync_info = mybir.SyncInfo(
                    on_wait=[rest[c0]], on_update=[])
                # remove from wherever the builder appended it
                for b2 in blocks:
                    for j in range(len(b2.instructions) - 1, -1, -1):
                        if b2.instructions[j] is ni:
                            b2.instructions.pop(j)
                            break
                nops.append(ni)
            for n_off, ni in enumerate(nops):
                b.instructions.insert(idx + n_off, ni)
            idx += len(nops) + 1


def _softmax_rowmajor(nc, pool, psum_l, out_sb, k):
    """Row-major softmax over free dim k. psum_l: [p, k] f32 logits;
    out_sb: [p, k] bf16 destination."""
    import concourse.mybir as mybir

    p = psum_l.shape[0]
    mx = pool.tile([p, 1], mybir.dt.float32, tag="smax_mx")
    sm = pool.tile([p, 1], mybir.dt.float32, tag="smax_sum")
    rc = pool.tile([p, 1], mybir.dt.float32, tag="smax_rcp")
    ex = pool.tile([p, k], mybir.dt.float32, tag="smax_exp")
    nc.vector.reduce_max(mx[:], psum_l[:], axis=mybir.AxisListType.X, negate=True)
    nc.scalar.activation(
        ex[:], psum_l[:], mybir.ActivationFunctionType.Exp,
        bias=mx[:], scale=1.0, accum_out=sm[:],
    )
    nc.vector.reciprocal(rc[:], sm[:])
    nc.vector.tensor_scalar_mul(out_sb[:], ex[:], rc[:])


def _build_program():
    import concourse.bass as bass
    import concourse.mybir as mybir
    import concourse.tile as tile

    _patch_tile_drain()
    f32 = mybir.dt.float32
    bf16 = mybir.dt.bfloat16
    fp8 = mybir.dt.float8e4

    nc = bass.Bass()

    # ---- DRAM inputs (per-core shards handed via in_maps) ----
    d_x8 = nc.dram_tensor("x8", [8, 128, 8, 304], fp8, kind="ExternalInput")
    d_at = nc.dram_tensor("at", [32, 128, 2, 1024], fp8, kind="ExternalInput")
    d_adiag = nc.dram_tensor("adiag", [128, 8, 128], bf16, kind="ExternalInput")
    d_adiagT = nc.dram_tensor("adiagT", [128, 8, 128], bf16, kind="ExternalInput")
    d_wcat0 = nc.dram_tensor("wcat0", [100, 3, 492], bf16, kind="ExternalInput")
    d_pWl0 = nc.dram_tensor("pWl0", [128, 2, 300], bf16, kind="ExternalInput")
    d_pWo0 = nc.dram_tensor("pWo0", [128, 3, K0], bf16, kind="ExternalInput")
    d_eWl0 = nc.dram_tensor("eWl0", [128, 3, 600], bf16, kind="ExternalInput")
    d_eWo0 = nc.dram_tensor("eWo0", [128, 5, 300], bf16, kind="ExternalInput")
    d_w1cat = nc.dram_tensor("w1cat", [128, 3, 450], bf16, kind="ExternalInput")
    d_pWl1 = nc.dram_tensor("pWl1", [75, 2, 300], bf16, kind="ExternalInput")
    d_pWo1 = nc.dram_tensor("pWo1", [100, 3, K1], bf16, kind="ExternalInput")
    d_eWl1 = nc.dram_tensor("eWl1", [100, 3, 600], bf16, kind="ExternalInput")
    d_eWo1 = nc.dram_tensor("eWo1", [120, 5, 300], bf16, kind="ExternalInput")
    d_eWh2 = nc.dram_tensor("eWh2", [100, 3, 300], bf16, kind="ExternalInput")
    d_eWl2 = nc.dram_tensor("eWl2", [100, 3, 600], bf16, kind="ExternalInput")
    d_eWo2 = nc.dram_tensor("eWo2", [120, 5, 300], bf16, kind="ExternalInput")
    d_lW1 = nc.dram_tensor("lW1", [100, 3, 600], bf16, kind="ExternalInput")
    d_lW2 = nc.dram_tensor("lW2", [120, 5, 128], bf16, kind="ExternalInput")
    d_lb1 = nc.dram_tensor("lb1", [120, 5], f32, kind="ExternalInput")
    d_lb2 = nc.dram_tensor("lb2", [128, 1], f32, kind="ExternalInput")
    d_ones = nc.dram_tensor("ones16", [64, GPC], bf16, kind="ExternalInput")
    d_s1mask = nc.dram_tensor("s1mask", [80, 2, 64], bf16, kind="ExternalInput")
    d_out = nc.dram_tensor("out", [128, GPC], f32, kind="ExternalOutput")

    with tile.TileContext(nc) as tc:
        with (
            tc.tile_pool(name="wpool", bufs=1) as wp,      # resident weights
            tc.tile_pool(name="big", bufs=1) as bigp,      # resident activations
            tc.tile_pool(name="atp", bufs=8) as atp,       # streamed AT tiles
            tc.tile_pool(name="tmp", bufs=4) as tmp,       # small temporaries
            tc.tile_pool(name="ps", bufs=8, space="PSUM") as psC,
        ):
            def load(dram, shape, eng=None):
                t = wp.tile(shape, dram.dtype, tag=dram.name)
                (eng or nc.scalar).dma_start(t[:], dram[:])
                return t

            # first at chunks + x8 chunks 0-1 lead the DMA order so the
            # aggregation matmuls can start immediately; remaining x8 chunks
            # are issued lazily inside the kk loop, weights go on scalar.
            # DMA-capable queues: sync (SP), scalar (Activation), gpsimd.
            x8 = wp.tile([128, 64, 304], fp8, tag="x8")
            dma_engs = [nc.sync, nc.gpsimd]

            def load_x8(nq):
                dma_engs[nq % 2].dma_start(
                    x8[:, nq * 8:(nq + 1) * 8, :], d_x8[nq])

            at_tiles = {}
            for c in (0, 1):
                at_t = atp.tile([128, 2, 1024], fp8, tag="at")
                dma_engs[c % 2].dma_start(at_t[:], d_at[c])
                at_tiles[c] = at_t
            load_x8(0)
            load_x8(1)
            # front-half weights only: everything else is deferred until
            # after the at stream so it doesn't steal HBM bandwidth from it
            wcat0 = load(d_wcat0, [100, 3, 492])
            pWl0 = load(d_pWl0, [128, 2, 300])
            pWo0 = load(d_pWo0, [128, 3, K0])
            eWl0 = load(d_eWl0, [128, 3, 600])
            eWo0 = load(d_eWo0, [128, 5, 300])

            Relu = mybir.ActivationFunctionType.Relu
            Copy = mybir.ActivationFunctionType.Copy
            CH150 = [(0, 128), (128, 150)]
            CH300 = [(0, 128), (128, 256), (256, 300)]
            CH492 = [(0, 128), (128, 256), (256, 384), (384, 492)]
            CH600 = [(0, 128), (128, 256), (256, 384), (384, 512), (512, 600)]

            # ---- stage B: MaggT = x^T @ AT  (feature-major aggregation) ----
            # A @ (x @ W) == (A @ x) @ W: aggregate the raw 300 features once,
            # then apply [pWh0|eWh0] to the [300, 1024] result.
            pss = [[psC.tile([100, 512], f32, tag="ps", name=f"psB_{nb}_{mi}")
                    for mi in range(3)] for nb in range(2)]
            for kk in range(0, 64, 2):
                c = kk // 2
                if kk % 8 == 0 and kk // 8 + 2 <= 7:
                    load_x8(kk // 8 + 2)
                if c in at_tiles:
                    at_t = at_tiles.pop(c)
                else:
                    at_t = atp.tile([128, 2, 1024], fp8, tag="at")
                    dma_engs[c % 2].dma_start(at_t[:], d_at[c])
                for nb in range(2):
                    for mi in range(3):
                        nc.tensor.matmul(
                            pss[nb][mi][:],
                            x8[:, kk:kk + 2, mi * 100:(mi + 1) * 100],
                            at_t[:, :, nb * 512:(nb + 1) * 512],
                            start=(kk == 0), stop=(kk == 62),
                            perf_mode=mybir.MatmulPerfMode.DoubleRow,
                        )
            # deferred weight loads: the at stream is done issuing, so these
            # no longer compete with it for HBM bandwidth
            adiag = load(d_adiag, [128, 8, 128], nc.sync)
            adiagT = load(d_adiagT, [128, 8, 128], nc.gpsimd)
            w1cat = load(d_w1cat, [128, 3, 450])
            pWl1 = load(d_pWl1, [75, 2, 300], nc.sync)
            pWo1 = load(d_pWo1, [100, 3, K1], nc.gpsimd)
            eWl1 = load(d_eWl1, [100, 3, 600])
            eWo1 = load(d_eWo1, [120, 5, 300], nc.sync)
            eWh2 = load(d_eWh2, [100, 3, 300], nc.gpsimd)
            eWl2 = load(d_eWl2, [100, 3, 600])
            eWo2 = load(d_eWo2, [120, 5, 300], nc.sync)
            lW1 = load(d_lW1, [100, 3, 600], nc.gpsimd)
            lW2 = load(d_lW2, [120, 5, 128])
            lb1 = load(d_lb1, [120, 5], nc.sync)
            lb2 = load(d_lb2, [128, 1], nc.gpsimd)
            ones16 = load(d_ones, [64, GPC])

            MaggT = bigp.tile([100, 3, 1024], bf16, tag="MaggT")
            for nb in range(2):
                for mi in range(3):
                    dst = MaggT[:, mi, nb * 512:(nb + 1) * 512]
                    if nb == 0:
                        nc.vector.tensor_copy(dst, pss[nb][mi][:])
                    else:
                        nc.scalar.activation(dst, pss[nb][mi][:], Copy)

            # ---- weight-apply: G = relu(wcat0^T @ MaggT), feature-major ----
            # wcat0 column layout: pool [0:150) pad [150:192) emb [192:492).
            # 128-wide matmul chunks keep FWL on; the relu of chunk 1 splits
            # into Gp1 (pool 128:150 + pad) and Ge0 (emb 0:64) so every
            # downstream matmul operand starts at partition 0.
            Gp0 = bigp.tile([128, 1024], bf16, tag="Gp0")
            Gp1 = bigp.tile([32, 1024], bf16, tag="Gp1")
            Ge0 = bigp.tile([64, 1024], bf16, tag="Ge0")
            Ge1 = bigp.tile([128, 1024], bf16, tag="Ge1")
            Ge2 = bigp.tile([108, 1024], bf16, tag="Ge2")
            for nb in range(2):
                nbs = slice(nb * 512, (nb + 1) * 512)
                for gi, (go, ge) in enumerate(CH492):
                    gsz = ge - go
                    ps = psC.tile([gsz, 512], f32, tag="ps", name="psG")
                    for kc in range(3):
                        nc.tensor.matmul(
                            ps[:], wcat0[:, kc, go:ge],
                            MaggT[:, kc, nbs],
                            start=(kc == 0), stop=(kc == 2),
                        )
                    if gi == 0:
                        nc.scalar.activation(Gp0[:, nbs], ps[:], Relu)
                    elif gi == 1:
                        nc.scalar.activation(Gp1[:, nbs], ps[0:32, :], Relu)
                        nc.scalar.activation(Ge0[:, nbs], ps[64:128, :], Relu)
                    elif gi == 2:
                        nc.scalar.activation(Ge1[:, nbs], ps[:], Relu)
                    else:
                        nc.scalar.activation(Ge2[:, nbs], ps[0:108, :], Relu)

            # ---- level-0 chains (pool/emb interleaved for PE density) ----
            H1p = bigp.tile([128, 3, 1024], bf16, tag="H1p")
            H1e = bigp.tile([128, 5, 1024], bf16, tag="H1e")
            S_bd = bigp.tile([128, 8, 160], bf16, tag="S_bd")
            nc.any.memzero(S_bd[:])
            for nb in range(2):
                nbs = slice(nb * 512, (nb + 1) * 512)
                for mc, (mo, me) in enumerate(CH600):
                    ps = psC.tile([me - mo, 512], f32, tag="ps", name="psH1e")
                    nc.tensor.matmul(
                        ps[:], eWl0[0:64, 0, mo:me],
                        Ge0[:, nbs], start=True, stop=False)
                    nc.tensor.matmul(
                        ps[:], eWl0[:, 1, mo:me],
                        Ge1[:, nbs], start=False, stop=False)
                    nc.tensor.matmul(
                        ps[:], eWl0[0:108, 2, mo:me],
                        Ge2[:, nbs], start=False, stop=True)
                    nc.scalar.activation(H1e[0:me - mo, mc, nbs], ps[:], Relu)
                    if mc < 3:
                        mo2, me2 = CH300[mc]
                        ps2 = psC.tile([me2 - mo2, 512], f32, tag="ps",
                                       name="psH1p")
                        nc.tensor.matmul(
                            ps2[:], pWl0[:, 0, mo2:me2],
                            Gp0[:, nbs], start=True, stop=False)
                        nc.tensor.matmul(
                            ps2[:], pWl0[0:32, 1, mo2:me2],
                            Gp1[:, nbs], start=False, stop=True)
                        nc.scalar.activation(
                            H1p[0:me2 - mo2, mc, nbs], ps2[:], Relu)

            # logits+softmax interleaved with Z
            Z = bigp.tile([128, 8, 300], bf16, tag="Z")
            for m in range(8):
                ps = psC.tile([128, K0], f32, tag="ps", name="psL")
                for kc, (ko, ke) in enumerate(CH300):
                    nc.tensor.matmul(
                        ps[:], H1p[0:ke - ko, kc, m * 128:(m + 1) * 128],
                        pWo0[0:ke - ko, kc, :],
                        start=(kc == 0), stop=(kc == 2),
                    )
                psz = psC.tile([128, 300], f32, tag="ps", name="psZ")
                for kc, (ko, ke) in enumerate(CH600):
                    nc.tensor.matmul(
                        psz[:], H1e[0:ke - ko, kc, m * 128:(m + 1) * 128],
                        eWo0[0:ke - ko, kc, :],
                        start=(kc == 0), stop=(kc == 4),
                    )
                if m % 2 == 0:
                    nc.vector.tensor_copy(Z[:, m, :], psz[:])
                else:
                    nc.scalar.activation(Z[:, m, :], psz[:], Copy)
                s_sb = tmp.tile([128, K0], bf16, tag="s0")
                _softmax_rowmajor(nc, tmp, ps, s_sb, K0)
                nc.gpsimd.tensor_copy(
                    S_bd[0:64, m, m * 20:m * 20 + 10], s_sb[0:64, :])
                nc.gpsimd.tensor_copy(
                    S_bd[64:128, m, m * 20 + 10:m * 20 + 20], s_sb[64:128, :])

            # ---- level-0 pooling ----
            # X1T[300, 160] = Z^T @ S_bd
            X1T = bigp.tile([128, 3, 160], bf16, tag="X1T")
            for mc, (mo, me) in enumerate(CH300):
                ps = psC.tile([me - mo, 160], f32, tag="ps", name="psX1T")
                for k in range(8):
                    nc.tensor.matmul(
                        ps[:], Z[:, k, mo:me], S_bd[:, k, :],
                        start=(k == 0), stop=(k == 7),
                    )
                nc.vector.tensor_copy(X1T[0:me - mo, mc, :], ps[:])

            # T_bd = A0_bd @ S_bd ; T2_bd = A0_bd^T @ S_bd  (block diag)
            T_bd = bigp.tile([128, 8, 160], bf16, tag="T_bd")
            T2_bd = bigp.tile([128, 8, 160], bf16, tag="T2_bd")
            nc.any.memzero(T_bd[:])
            nc.any.memzero(T2_bd[:])
            for c in range(8):
                psT = psC.tile([128, 20], f32, tag="ps", name="psT")
                nc.tensor.matmul(psT[:], adiagT[:, c, :],
                                 S_bd[:, c, c * 20:c * 20 + 20],
                                 start=True, stop=True)
                nc.vector.tensor_copy(T_bd[:, c, c * 20:c * 20 + 20], psT[:])
                psT2 = psC.tile([128, 20], f32, tag="ps", name="psT2")
                nc.tensor.matmul(psT2[:], adiag[:, c, :],
                                 S_bd[:, c, c * 20:c * 20 + 20],
                                 start=True, stop=True)
                nc.vector.tensor_copy(T2_bd[:, c, c * 20:c * 20 + 20], psT2[:])

            # A1_bd = S_bd^T @ T_bd ; A1T_bd = S_bd^T @ T2_bd   [160, 160]
            A1bd = bigp.tile([80, 2, 160], bf16, tag="A1bd")
            A1Tbd = bigp.tile([80, 2, 160], bf16, tag="A1Tbd")
            for mc in range(2):
                ps1 = psC.tile([80, 160], f32, tag="ps", name="psA1")
                ps2 = psC.tile([80, 160], f32, tag="ps", name="psA1T")
                for k in range(8):
                    nc.tensor.matmul(
                        ps1[:], S_bd[:, k, mc * 80:(mc + 1) * 80], T_bd[:, k, :],
                        start=(k == 0), stop=(k == 7))
                for k in range(8):
                    nc.tensor.matmul(
                        ps2[:], S_bd[:, k, mc * 80:(mc + 1) * 80], T2_bd[:, k, :],
                        start=(k == 0), stop=(k == 7))
                nc.vector.tensor_copy(A1bd[:, mc, :], ps1[:])
                nc.vector.tensor_copy(A1Tbd[:, mc, :], ps2[:])

            # ---- level 1 ----
            # Y1pe [160, 450] row-major = X1 @ [pWh1|eWh1]
            Y1pe = bigp.tile([80, 2, 450], bf16, tag="Y1pe")
            for mi in range(2):
                pse = psC.tile([80, 450], f32, tag="ps", name="psY1pe")
                for kc, (ko, ke) in enumerate(CH300):
                    nc.tensor.matmul(
                        pse[:], X1T[0:ke - ko, kc, mi * 80:(mi + 1) * 80],
                        w1cat[0:ke - ko, kc, :],
                        start=(kc == 0), stop=(kc == 2))
                nc.vector.tensor_copy(Y1pe[:, mi, :], pse[:])

            # M1pt [150, 160] = Y1p^T @ A1T_bd, relu -> G1p [75, 2, 160]
            G1p = bigp.tile([75, 2, 160], bf16, tag="G1p")
            for mf in range(2):
                ps = psC.tile([75, 160], f32, tag="ps", name="psM1p")
                for kc in range(2):
                    nc.tensor.matmul(
                        ps[:], Y1pe[:, kc, mf * 75:(mf + 1) * 75],
                        A1Tbd[:, kc, :],
                        start=(kc == 0), stop=(kc == 1))
                nc.scalar.activation(G1p[:, mf, :], ps[:], Relu)

            G1e = bigp.tile([100, 3, 160], bf16, tag="G1e")
            for mf in range(3):
                ps = psC.tile([100, 160], f32, tag="ps", name="psM1e")
                for kc in range(2):
                    nc.tensor.matmul(
                        ps[:], Y1pe[:, kc, 150 + mf * 100:150 + (mf + 1) * 100],
                        A1Tbd[:, kc, :],
                        start=(kc == 0), stop=(kc == 1))
                nc.scalar.activation(G1e[:, mf, :], ps[:], Relu)

            # pool chain level 1
            H1p1 = bigp.tile([100, 3, 160], bf16, tag="H1p1")
            for mc in range(3):
                ps = psC.tile([100, 160], f32, tag="ps", name="psH1p1")
                for kc in range(2):
                    nc.tensor.matmul(
                        ps[:], pWl1[:, kc, mc * 100:(mc + 1) * 100], G1p[:, kc, :],
                        start=(kc == 0), stop=(kc == 1))
                nc.scalar.activation(H1p1[:, mc, :], ps[:], Relu)

            H1e1 = bigp.tile([120, 5, 160], bf16, tag="H1e1")
            for mc in range(5):
                ps = psC.tile([120, 160], f32, tag="ps", name="psH1e1")
                for kc in range(3):
                    nc.tensor.matmul(
                        ps[:], eWl1[:, kc, mc * 120:(mc + 1) * 120], G1e[:, kc, :],
                        start=(kc == 0), stop=(kc == 2))
                nc.scalar.activation(H1e1[:, mc, :], ps[:], Relu)

            S1_bd = bigp.tile([80, 2, 64], bf16, tag="S1_bd")
            s1mask = load(d_s1mask, [80, 2, 64])
            for mi in range(2):
                ps = psC.tile([80, K1], f32, tag="ps", name="psL1")
                for kc in range(3):
                    nc.tensor.matmul(
                        ps[:], H1p1[:, kc, mi * 80:(mi + 1) * 80], pWo1[:, kc, :],
                        start=(kc == 0), stop=(kc == 2))
                s_sb = tmp.tile([80, K1], bf16, tag="s1")
                _softmax_rowmajor(nc, tmp, ps, s_sb, K1)
                # block-diag scatter: replicate the [80,4] softmax 16x along
                # free dim and mask to the owning graph's 4 columns
                nc.vector.tensor_tensor(
                    S1_bd[:, mi, :].rearrange("p (b j) -> p b j", j=K1),
                    s_sb[:, None, :].to_broadcast((80, GPC, K1)),
                    s1mask[:, mi, :].rearrange("p (b j) -> p b j", j=K1),
                    mybir.AluOpType.mult)

            Z1 = bigp.tile([80, 2, 300], bf16, tag="Z1")
            for mi in range(2):
                ps = psC.tile([80, 300], f32, tag="ps", name="psZ1")
                for kc in range(5):
                    nc.tensor.matmul(
                        ps[:], H1e1[:, kc, mi * 80:(mi + 1) * 80], eWo1[:, kc, :],
                        start=(kc == 0), stop=(kc == 4))
                nc.vector.tensor_copy(Z1[:, mi, :], ps[:])

            # pooling level 1
            X2T = bigp.tile([100, 3, 64], bf16, tag="X2T")
            for mc in range(3):
                ps = psC.tile([100, 64], f32, tag="ps", name="psX2T")
                for kc in range(2):
                    nc.tensor.matmul(
                        ps[:], Z1[:, kc, mc * 100:(mc + 1) * 100], S1_bd[:, kc, :],
                        start=(kc == 0), stop=(kc == 1))
                nc.vector.tensor_copy(X2T[:, mc, :], ps[:])

            # T3 = A1_bd^T @ S1_bd ; A2T_bd = S1_bd^T @ T3   [64, 64]
            T3 = bigp.tile([80, 2, 64], bf16, tag="T3")
            for mi in range(2):
                ps = psC.tile([80, 64], f32, tag="ps", name="psT3")
                for kc in range(2):
                    nc.tensor.matmul(
                        ps[:], A1bd[:, kc, mi * 80:(mi + 1) * 80], S1_bd[:, kc, :],
                        start=(kc == 0), stop=(kc == 1))
                nc.vector.tensor_copy(T3[:, mi, :], ps[:])
            A2Tbd = bigp.tile([64, 64], bf16, tag="A2Tbd")
            psA2 = psC.tile([64, 64], f32, tag="ps", name="psA2T")
            for kc in range(2):
                nc.tensor.matmul(
                    psA2[:], S1_bd[:, kc, :], T3[:, kc, :],
                    start=(kc == 0), stop=(kc == 1))
            nc.vector.tensor_copy(A2Tbd[:], psA2[:])

            # ---- level 2 (emb only; S2 == 1) ----
            Y2 = bigp.tile([64, 300], bf16, tag="Y2")
            psY2 = psC.tile([64, 300], f32, tag="ps", name="psY2")
            for kc in range(3):
                nc.tensor.matmul(
                    psY2[:], X2T[:, kc, 0:64], eWh2[:, kc, :],
                    start=(kc == 0), stop=(kc == 2))
            nc.vector.tensor_copy(Y2[:], psY2[:])

            G2 = bigp.tile([100, 3, 64], bf16, tag="G2")
            for mf in range(3):
                ps = psC.tile([100, 64], f32, tag="ps", name="psM2")
                nc.tensor.matmul(
                    ps[:], Y2[:, mf * 100:(mf + 1) * 100], A2Tbd[:],
                    start=True, stop=True)
                nc.scalar.activation(G2[:, mf, :], ps[:], Relu)

            H2 = bigp.tile([120, 5, 64], bf16, tag="H2")
            for mc in range(5):
                ps = psC.tile([120, 64], f32, tag="ps", name="psH2")
                for kc in range(3):
                    nc.tensor.matmul(
                        ps[:], eWl2[:, kc, mc * 120:(mc + 1) * 120], G2[:, kc, :],
                        start=(kc == 0), stop=(kc == 2))
                nc.scalar.activation(H2[:, mc, :], ps[:], Relu)

            Z2 = bigp.tile([64, 300], bf16, tag="Z2")
            psZ2 = psC.tile([64, 300], f32, tag="ps", name="psZ2")
            for kc in range(5):
                nc.tensor.matmul(
                    psZ2[:], H2[:, kc, 0:64], eWo2[:, kc, :],
                    start=(kc == 0), stop=(kc == 4))
            nc.vector.tensor_copy(Z2[:], psZ2[:])

            # X3T [300, 16] = Z2^T @ ones_bd
            X3T = bigp.tile([100, 3, GPC], bf16, tag="X3T")
            for mf in range(3):
                ps = psC.tile([100, GPC], f32, tag="ps", name="psX3T")
                nc.tensor.matmul(
                    ps[:], Z2[:, mf * 100:(mf + 1) * 100], ones16[:],
                    start=True, stop=True)
                nc.vector.tensor_copy(X3T[:, mf, :], ps[:])

            # ---- head ----
            hT = bigp.tile([120, 5, GPC], bf16, tag="hT")
            for mc in range(5):
                ps = psC.tile([120, GPC], f32, tag="ps", name="psh")
                for kc in range(3):
                    nc.tensor.matmul(
                        ps[:], lW1[:, kc, mc * 120:(mc + 1) * 120], X3T[:, kc, :],
                        start=(kc == 0), stop=(kc == 2))
                nc.scalar.activation(hT[:, mc, :], ps[:], Relu,
                                     bias=lb1[:, mc:mc + 1])

            psO = psC.tile([128, GPC], f32, tag="ps", name="psO")
            for kc in range(5):
                nc.tensor.matmul(
                    psO[:], lW2[:, kc, :], hT[:, kc, :],
                    start=(kc == 0), stop=(kc == 4))
            outT = tmp.tile([128, GPC], f32, tag="outT")
            nc.vector.tensor_scalar_add(outT[:], psO[:], lb2[:])
            nc.sync.dma_start(d_out[:], outT[:])

    _split_excess_waits(nc)
    return nc


def _host_prep(inputs):
    """Build per-core in_maps from the full inputs."""
    ONE = np.uint8(0x38)  # 1.0 in float8_e4m3

    x = np.asarray(inputs["x"], np.float32)
    ei = np.asarray(inputs["edge_index"]).astype(np.int64)

    # full A^T in bf16 bit pattern: AT[j, i] = A[i, j]
    ATu = np.zeros((N_NODES, N_NODES), np.uint8)
    ATu[ei[1], ei[0]] = ONE

    # x8[q][p, m, f] = x[128*(8q+m) + p, f], padded 300 -> 304 so the
    # DoubleRow Ko step (304 B) stays 16-byte aligned
    xp = np.zeros((N_NODES, 304), np.float32)
    xp[:, 0:300] = x
    x8 = np.ascontiguousarray(
        xp.reshape(8, 8, 128, 304).transpose(0, 2, 1, 3)).astype(F8)

    def chunkw(w, p, c):
        w = np.asarray(w, np.float32)
        return np.ascontiguousarray(
            w.reshape(c, p, -1).transpose(1, 0, 2)).astype(BF)

    def padchunk(w, rowchunks, c, m):
        w = np.asarray(w, np.float32)
        out = np.zeros((128, c, m), np.float32)
        for ci, (a, b) in enumerate(rowchunks):
            out[0:b - a, ci, :] = w[a:b, :]
        return out.astype(BF)

    wcat0 = np.zeros((300, 492), np.float32)
    wcat0[:, 0:150] = np.asarray(inputs["pWh0"], np.float32)
    wcat0[:, 192:492] = np.asarray(inputs["eWh0"], np.float32)

    ones16 = np.zeros((64, GPC), BF)
    for b in range(GPC):
        ones16[b * 4:(b + 1) * 4, b] = 1
    s1mask = np.zeros((80, 2, 64), BF)
    for mi in range(2):
        for p in range(80):
            gb = (80 * mi + p) // K1NODES
            s1mask[p, mi, gb * 4:(gb + 1) * 4] = 1
    lb1 = np.ascontiguousarray(
        np.asarray(inputs["lb1"], np.float32).reshape(5, 120).T)
    lb2 = np.asarray(inputs["lb2"], np.float32).reshape(128, 1)

    CH300 = [(0, 128), (128, 256), (256, 300)]
    CH600 = [(0, 128), (128, 256), (256, 384), (384, 512), (512, 600)]
    w1cat = np.concatenate([np.asarray(inputs["pWh1"], np.float32),
                            np.asarray(inputs["eWh1"], np.float32)], axis=1)

    shared = {
        "x8": x8,
        "wcat0": chunkw(wcat0, 100, 3),
        "pWl0": padchunk(inputs["pWl0"], [(0, 128), (128, 150)], 2, 300),
        "pWo0": padchunk(inputs["pWo0"], CH300, 3, K0),
        "eWl0": padchunk(inputs["eWl0"], [(0, 64), (64, 192), (192, 300)], 3, 600),
        "eWo0": padchunk(inputs["eWo0"], CH600, 5, 300),
        "w1cat": padchunk(w1cat, CH300, 3, 450),
        "pWl1": chunkw(inputs["pWl1"], 75, 2),
        "pWo1": chunkw(inputs["pWo1"], 100, 3),
        "eWl1": chunkw(inputs["eWl1"], 100, 3),
        "eWo1": chunkw(inputs["eWo1"], 120, 5),
        "eWh2": chunkw(inputs["eWh2"], 100, 3),
        "eWl2": chunkw(inputs["eWl2"], 100, 3),
        "eWo2": chunkw(inputs["eWo2"], 120, 5),
        "lW1": chunkw(inputs["lW1"], 100, 3),
        "lW2": chunkw(inputs["lW2"], 120, 5),
        "lb1": lb1,
        "lb2": lb2,
        "ones16": ones16,
        "s1mask": s1mask,
    }

    in_maps = []
    for d in range(N_CORES):
        r0 = d * R
        slab = ATu[:, r0:r0 + R]  # [8192, 1024]
        at = np.ascontiguousarray(
            slab.reshape(32, 2, 128, 1024).transpose(0, 2, 1, 3)).view(F8)

        adiag = np.zeros((128, 8, 128), np.uint8)
        adiagT = np.zeros((128, 8, 128), np.uint8)
        for c in range(8):
            # full 128x128 slab block, then mask to per-graph 64x64 diag
            blkT = slab[r0 + 128 * c: r0 + 128 * (c + 1),
                        128 * c: 128 * (c + 1)]  # blkT[q, p] = A[rows p, cols q]
            blk = blkT.T
            for h in range(2):
                s = slice(64 * h, 64 * (h + 1))
                adiag[s, c, s] = blk[s, s]
                adiagT[s, c, s] = blkT[s, s]
        m = dict(shared)
        m["at"] = at
        m["adiag"] = adiag.view(F8).astype(BF)
        m["adiagT"] = adiagT.view(F8).astype(BF)
        in_maps.append(m)
    return in_maps


def _run(inputs, trace=False, trace_kwargs=None):
    try:
        import concourse.bass as bass  # noqa: F401
    except ImportError:
        import sys
        sys.path.insert(0, "/opt/trn_rl_repo")
    from concourse.bass_utils import run_bass_kernel_spmd

    if "prog" not in _prog_cache:
        _prog_cache["prog"] = _build_program()
    nc = _prog_cache["prog"]

    in_maps = _host_prep(inputs)
    res = run_bass_kernel_spmd(
        nc, in_maps, core_ids=list(range(N_CORES)), trace=trace,
        **(trace_kwargs or {}),
    )
    out = np.empty((B, 128), np.float32)
    for d in range(N_CORES):
        out[d * GPC:(d + 1) * GPC, :] = res.results[d]["out"].T
    return out, res


def kernel(**inputs):
    out, _ = _run(inputs, trace=False)
    return out



# revision 21
# speedup vs baseline: 1.3533x; 1.0381x over previous
"""DiffPool GNN encoder on 8 Trainium2 NeuronCores.

Data-parallel over graphs: core d owns graphs [16d, 16d+16) = node rows
[1024d, 1024d+1024). Host builds each core's dense A^T slab (bf16 0/1),
the per-graph block-diagonal 64x64 A blocks, x^T, and pre-chunked bf16
weights. The device kernel computes, per core:

  level 0:  MaggT = x^T @ AT_slab          (feature-major aggregation;
            A @ (x@W) == (A@x) @ W so raw x (fp8) is aggregated once)
            G = relu([pWh0|eWh0]^T @ MaggT)
            pool chain -> softmax S0 -> block-diag S_bd [1024,160]
            emb chain  -> Z [1024,300] row-major
            X1T = Z^T @ S_bd, A1 = S^T A S (+ transposed variant), both
            via block-diag matmuls
  level 1:  same, 16 graphs x 10 nodes
  level 2:  emb only (pool softmax over k=1 is identically 1), X3 = per
            graph column sum of Z2
  head:     out^T = lW2^T @ relu(lW1^T @ X3T + lb1) + lb2   [128, 16]

Host gathers the 8 [128,16] outputs into the [128,128] result.
"""

import numpy as np
import ml_dtypes

BF = ml_dtypes.bfloat16
F8 = ml_dtypes.float8_e4m3fn
N_CORES = 8
N_NODES = 8192
B = 128
GPC = 16          # graphs per core
R = 1024          # rows per core
N0 = 64           # nodes per graph at level 0
D = 300
K0, K1 = 10, 4
K1NODES = 10  # nodes per graph at level 1

_prog_cache = {}


def _patch_tile_drain():
    """This container's walrus rejects >2 sync waits on one instruction;
    split the Tile tail-drain waits across several drains."""
    import concourse.tile as tile_mod
    from concourse.vector_clock import ScopedClock, VectorClock

    if getattr(tile_mod.TileContext, "_drain_patched", False):
        return

    def _patched(self, tick_clock, wait_clock):
        gc = tick_clock.global_clock
        n = len(gc)
        for start in range(0, n, 2):
            partial = VectorClock(
                [gc[p] if start <= p < start + 2 else 0 for p in range(n)]
            )
            di = self.nc.sync.drain()
            wait_clock.add_sem_waits(di.ins, ScopedClock({None: partial}))
        self.nc.all_engine_barrier()
        assert self.sems is not None
        popped = self.nc._tile_sem_poison_stack.pop()
        assert popped is self._sem_poison
        self.nc.clear_and_free_semaphores(list(self.sems.allocated().values()))
        self.nc.all_engine_barrier()

    tile_mod.TileContext._drain_and_barrier = _patched
    tile_mod.TileContext._drain_patched = True


def _split_excess_waits(nc, max_waits=1):
    """walrus here rejects instructions with >2 sync waits. Move excess waits
    onto injected same-engine nops placed immediately before the instruction
    (engine queues execute in order, so this preserves semantics)."""
    import concourse.mybir as mybir

    blocks = nc.m.functions[0].blocks
    for b in blocks:
        idx = 0
        while idx < len(b.instructions):
            inst = b.instructions[idx]
            si = inst.sync_info
            lim = max_waits
            if si is None or not si.on_wait or len(si.on_wait) <= lim:
                idx += 1
                continue
            waits = list(si.on_wait)
            keep = waits[-lim:]
            rest = waits[:-lim]
            inst.sync_info = mybir.SyncInfo(
                on_wait=keep, on_update=list(si.on_update or []))
            nops = []
            for c0 in range(0, len(rest)):
                n = nc.engines[inst.engine].nop(nofuse=True)
                ni = n.ins
                ni.sync_info = mybir.SyncInfo(
                    on_wait=[rest[c0]], on_update=[])
                # remove from wherever the builder appended it
                for b2 in blocks:
                    for j in range(len(b2.instructions) - 1, -1, -1):
                        if b2.instructions[j] is ni:
                            b2.instructions.pop(j)
                            break
                nops.append(ni)
            for n_off, ni in enumerate(nops):
                b.instructions.insert(idx + n_off, ni)
            idx += len(nops) + 1


def _softmax_rowmajor(nc, pool, psum_l, out_sb, k):
    """Row-major softmax over free dim k. psum_l: [p, k] f32 logits;
    out_sb: [p, k] bf16 destination."""
    import concourse.mybir as mybir

    p = psum_l.shape[0]
    mx = pool.tile([p, 1], mybir.dt.float32, tag="smax_mx")
    sm = pool.tile([p, 1], mybir.dt.float32, tag="smax_sum")
    rc = pool.tile([p, 1], mybir.dt.float32, tag="smax_rcp")
    ex = pool.tile([p, k], mybir.dt.float32, tag="smax_exp")
    nc.vector.reduce_max(mx[:], psum_l[:], axis=mybir.AxisListType.X, negate=True)
    nc.scalar.activation(
        ex[:], psum_l[:], mybir.ActivationFunctionType.Exp,
        bias=mx[:], scale=1.0, accum_out=sm[:],
    )
    nc.vector.reciprocal(rc[:], sm[:])
    nc.vector.tensor_scalar_mul(out_sb[:], ex[:], rc[:])


def _build_program():
    import concourse.bass as bass
    import concourse.mybir as mybir
    import concourse.tile as tile

    _patch_tile_drain()
    f32 = mybir.dt.float32
    bf16 = mybir.dt.bfloat16
    fp8 = mybir.dt.float8e4

    nc = bass.Bass()

    # ---- DRAM inputs (per-core shards handed via in_maps) ----
    d_x8 = nc.dram_tensor("x8", [8, 128, 8, 304], fp8, kind="ExternalInput")
    d_at = nc.dram_tensor("at", [32, 128, 2, 1024], fp8, kind="ExternalInput")
    d_adiag = nc.dram_tensor("adiag", [128, 8, 128], bf16, kind="ExternalInput")
    d_adiagT = nc.dram_tensor("adiagT", [128, 8, 128], bf16, kind="ExternalInput")
    d_wcat0 = nc.dram_tensor("wcat0", [100, 3, 492], bf16, kind="ExternalInput")
    d_pWl0 = nc.dram_tensor("pWl0", [128, 2, 300], bf16, kind="ExternalInput")
    d_pWo0 = nc.dram_tensor("pWo0", [128, 3, K0], bf16, kind="ExternalInput")
    d_eWl0 = nc.dram_tensor("eWl0", [128, 3, 600], bf16, kind="ExternalInput")
    d_eWo0 = nc.dram_tensor("eWo0", [128, 5, 300], bf16, kind="ExternalInput")
    d_w1cat = nc.dram_tensor("w1cat", [128, 3, 450], bf16, kind="ExternalInput")
    d_pWl1 = nc.dram_tensor("pWl1", [75, 2, 300], bf16, kind="ExternalInput")
    d_pWo1 = nc.dram_tensor("pWo1", [100, 3, K1], bf16, kind="ExternalInput")
    d_eWl1 = nc.dram_tensor("eWl1", [100, 3, 600], bf16, kind="ExternalInput")
    d_eWo1 = nc.dram_tensor("eWo1", [120, 5, 300], bf16, kind="ExternalInput")
    d_eWh2 = nc.dram_tensor("eWh2", [100, 3, 300], bf16, kind="ExternalInput")
    d_eWl2 = nc.dram_tensor("eWl2", [100, 3, 600], bf16, kind="ExternalInput")
    d_eWo2 = nc.dram_tensor("eWo2", [120, 5, 300], bf16, kind="ExternalInput")
    d_lW1 = nc.dram_tensor("lW1", [100, 3, 600], bf16, kind="ExternalInput")
    d_lW2 = nc.dram_tensor("lW2", [120, 5, 128], bf16, kind="ExternalInput")
    d_lb1 = nc.dram_tensor("lb1", [120, 5], f32, kind="ExternalInput")
    d_lb2 = nc.dram_tensor("lb2", [128, 1], f32, kind="ExternalInput")
    d_ones = nc.dram_tensor("ones16", [64, GPC], bf16, kind="ExternalInput")
    d_s1mask = nc.dram_tensor("s1mask", [80, 2, 64], bf16, kind="ExternalInput")
    d_out = nc.dram_tensor("out", [128, GPC], f32, kind="ExternalOutput")

    with tile.TileContext(nc) as tc:
        with (
            tc.tile_pool(name="wpool", bufs=1) as wp,      # resident weights
            tc.tile_pool(name="big", bufs=1) as bigp,      # resident activations
            tc.tile_pool(name="atp", bufs=8) as atp,       # streamed AT tiles
            tc.tile_pool(name="tmp", bufs=4) as tmp,       # small temporaries
            tc.tile_pool(name="ps", bufs=8, space="PSUM") as psC,
        ):
            def load(dram, shape, eng=None, gate=None):
                t = wp.tile(shape, dram.dtype, tag=dram.name)
                if gate is not None:
                    # WAW hazard: 1-element dummy write that depends on the
                    # gate tile, forcing the scheduler to start this DMA only
                    # after the gate is produced (keeps deferred weight loads
                    # from stealing HBM bandwidth during the at stream).
                    idx = (slice(0, 1),) + (0,) * (len(shape) - 2) + (slice(0, 1),)
                    nc.vector.tensor_copy(t[idx], gate)
                (eng or nc.scalar).dma_start(t[:], dram[:])
                return t

            # first at chunks + x8 chunks 0-1 lead the DMA order so the
            # aggregation matmuls can start immediately; remaining x8 chunks
            # are issued lazily inside the kk loop, weights go on scalar.
            # DMA-capable queues: sync (SP), scalar (Activation), gpsimd.
            x8 = wp.tile([128, 64, 304], fp8, tag="x8")
            dma_engs = [nc.sync, nc.gpsimd]

            def load_x8(nq):
                dma_engs[nq % 2].dma_start(
                    x8[:, nq * 8:(nq + 1) * 8, :], d_x8[nq])

            at_tiles = {}
            for c in (0, 1):
                at_t = atp.tile([128, 2, 1024], fp8, tag="at")
                dma_engs[c % 2].dma_start(at_t[:], d_at[c])
                at_tiles[c] = at_t
            load_x8(0)
            load_x8(1)
            # front-half weights only: everything else is deferred until
            # after the at stream so it doesn't steal HBM bandwidth from it
            wcat0 = load(d_wcat0, [100, 3, 492])
            pWl0 = load(d_pWl0, [128, 2, 300])
            pWo0 = load(d_pWo0, [128, 3, K0])
            eWl0 = load(d_eWl0, [128, 3, 600])
            eWo0 = load(d_eWo0, [128, 5, 300])

            Relu = mybir.ActivationFunctionType.Relu
            Copy = mybir.ActivationFunctionType.Copy
            CH150 = [(0, 128), (128, 150)]
            CH300 = [(0, 128), (128, 256), (256, 300)]
            CH492 = [(0, 128), (128, 256), (256, 384), (384, 492)]
            CH600 = [(0, 128), (128, 256), (256, 384), (384, 512), (512, 600)]

            # ---- stage B: MaggT = x^T @ AT  (feature-major aggregation) ----
            # A @ (x @ W) == (A @ x) @ W: aggregate the raw 300 features once,
            # then apply [pWh0|eWh0] to the [300, 1024] result.
            pss = [[psC.tile([100, 512], f32, tag="ps", name=f"psB_{nb}_{mi}")
                    for mi in range(3)] for nb in range(2)]
            dma_engs3 = [nc.sync, nc.gpsimd, nc.scalar]
            for kk in range(0, 64, 2):
                c = kk // 2
                if kk % 8 == 0 and kk // 8 + 2 <= 7:
                    load_x8(kk // 8 + 2)
                if c in at_tiles:
                    at_t = at_tiles.pop(c)
                else:
                    at_t = atp.tile([128, 2, 1024], fp8, tag="at")
                    dma_engs3[c % 3].dma_start(at_t[:], d_at[c])
                for nb in range(2):
                    for mi in range(3):
                        nc.tensor.matmul(
                            pss[nb][mi][:],
                            x8[:, kk:kk + 2, mi * 100:(mi + 1) * 100],
                            at_t[:, :, nb * 512:(nb + 1) * 512],
                            start=(kk == 0), stop=(kk == 62),
                            perf_mode=mybir.MatmulPerfMode.DoubleRow,
                        )
            # deferred weight loads, gated on the last at chunk's arrival so
            # they don't compete with the at stream for HBM bandwidth
            gate = at_t[0:1, 0, 0:1]
            adiag = load(d_adiag, [128, 8, 128], nc.sync, gate)
            adiagT = load(d_adiagT, [128, 8, 128], nc.gpsimd, gate)
            w1cat = load(d_w1cat, [128, 3, 450], None, gate)
            pWl1 = load(d_pWl1, [75, 2, 300], nc.sync, gate)
            pWo1 = load(d_pWo1, [100, 3, K1], nc.gpsimd, gate)
            eWl1 = load(d_eWl1, [100, 3, 600], None, gate)
            eWo1 = load(d_eWo1, [120, 5, 300], nc.sync, gate)
            eWh2 = load(d_eWh2, [100, 3, 300], nc.gpsimd, gate)
            eWl2 = load(d_eWl2, [100, 3, 600], None, gate)
            eWo2 = load(d_eWo2, [120, 5, 300], nc.sync, gate)
            lW1 = load(d_lW1, [100, 3, 600], nc.gpsimd, gate)
            lW2 = load(d_lW2, [120, 5, 128], None, gate)
            lb1 = load(d_lb1, [120, 5], nc.sync, gate)
            lb2 = load(d_lb2, [128, 1], nc.gpsimd, gate)
            ones16 = load(d_ones, [64, GPC], None, gate)

            MaggT = bigp.tile([100, 3, 1024], bf16, tag="MaggT")
            for nb in range(2):
                for mi in range(3):
                    dst = MaggT[:, mi, nb * 512:(nb + 1) * 512]
                    if nb == 0:
                        nc.vector.tensor_copy(dst, pss[nb][mi][:])
                    else:
                        nc.scalar.activation(dst, pss[nb][mi][:], Copy)

            # ---- weight-apply: G = relu(wcat0^T @ MaggT), feature-major ----
            # wcat0 column layout: pool [0:150) pad [150:192) emb [192:492).
            # 128-wide matmul chunks keep FWL on; the relu of chunk 1 splits
            # into Gp1 (pool 128:150 + pad) and Ge0 (emb 0:64) so every
            # downstream matmul operand starts at partition 0.
            Gp0 = bigp.tile([128, 1024], bf16, tag="Gp0")
            Gp1 = bigp.tile([32, 1024], bf16, tag="Gp1")
            Ge0 = bigp.tile([64, 1024], bf16, tag="Ge0")
            Ge1 = bigp.tile([128, 1024], bf16, tag="Ge1")
            Ge2 = bigp.tile([108, 1024], bf16, tag="Ge2")
            for nb in range(2):
                nbs = slice(nb * 512, (nb + 1) * 512)
                for gi, (go, ge) in enumerate(CH492):
                    gsz = ge - go
                    ps = psC.tile([gsz, 512], f32, tag="ps", name="psG")
                    for kc in range(3):
                        nc.tensor.matmul(
                            ps[:], wcat0[:, kc, go:ge],
                            MaggT[:, kc, nbs],
                            start=(kc == 0), stop=(kc == 2),
                        )
                    if gi == 0:
                        nc.scalar.activation(Gp0[:, nbs], ps[:], Relu)
                    elif gi == 1:
                        nc.scalar.activation(Gp1[:, nbs], ps[0:32, :], Relu)
                        nc.scalar.activation(Ge0[:, nbs], ps[64:128, :], Relu)
                    elif gi == 2:
                        nc.scalar.activation(Ge1[:, nbs], ps[:], Relu)
                    else:
                        nc.scalar.activation(Ge2[:, nbs], ps[0:108, :], Relu)

            # ---- level-0 chains (pool/emb interleaved for PE density) ----
            H1p = bigp.tile([128, 3, 1024], bf16, tag="H1p")
            H1e = bigp.tile([128, 5, 1024], bf16, tag="H1e")
            S_bd = bigp.tile([128, 8, 160], bf16, tag="S_bd")
            nc.any.memzero(S_bd[:])
            for nb in range(2):
                nbs = slice(nb * 512, (nb + 1) * 512)
                for mc, (mo, me) in enumerate(CH600):
                    ps = psC.tile([me - mo, 512], f32, tag="ps", name="psH1e")
                    nc.tensor.matmul(
                        ps[:], eWl0[0:64, 0, mo:me],
                        Ge0[:, nbs], start=True, stop=False)
                    nc.tensor.matmul(
                        ps[:], eWl0[:, 1, mo:me],
                        Ge1[:, nbs], start=False, stop=False)
                    nc.tensor.matmul(
                        ps[:], eWl0[0:108, 2, mo:me],
                        Ge2[:, nbs], start=False, stop=True)
                    nc.scalar.activation(H1e[0:me - mo, mc, nbs], ps[:], Relu)
                    if mc < 3:
                        mo2, me2 = CH300[mc]
                        ps2 = psC.tile([me2 - mo2, 512], f32, tag="ps",
                                       name="psH1p")
                        nc.tensor.matmul(
                            ps2[:], pWl0[:, 0, mo2:me2],
                            Gp0[:, nbs], start=True, stop=False)
                        nc.tensor.matmul(
                            ps2[:], pWl0[0:32, 1, mo2:me2],
                            Gp1[:, nbs], start=False, stop=True)
                        nc.scalar.activation(
                            H1p[0:me2 - mo2, mc, nbs], ps2[:], Relu)

            # logits+softmax interleaved with Z
            Z = bigp.tile([128, 8, 300], bf16, tag="Z")
            for m in range(8):
                ps = psC.tile([128, K0], f32, tag="ps", name="psL")
                for kc, (ko, ke) in enumerate(CH300):
                    nc.tensor.matmul(
                        ps[:], H1p[0:ke - ko, kc, m * 128:(m + 1) * 128],
                        pWo0[0:ke - ko, kc, :],
                        start=(kc == 0), stop=(kc == 2),
                    )
                psz = psC.tile([128, 300], f32, tag="ps", name="psZ")
                for kc, (ko, ke) in enumerate(CH600):
                    nc.tensor.matmul(
                        psz[:], H1e[0:ke - ko, kc, m * 128:(m + 1) * 128],
                        eWo0[0:ke - ko, kc, :],
                        start=(kc == 0), stop=(kc == 4),
                    )
                if m % 2 == 0:
                    nc.vector.tensor_copy(Z[:, m, :], psz[:])
                else:
                    nc.scalar.activation(Z[:, m, :], psz[:], Copy)
                s_sb = tmp.tile([128, K0], bf16, tag="s0")
                _softmax_rowmajor(nc, tmp, ps, s_sb, K0)
                nc.gpsimd.tensor_copy(
                    S_bd[0:64, m, m * 20:m * 20 + 10], s_sb[0:64, :])
                nc.gpsimd.tensor_copy(
                    S_bd[64:128, m, m * 20 + 10:m * 20 + 20], s_sb[64:128, :])

            # ---- level-0 pooling ----
            # X1T[300, 160] = Z^T @ S_bd
            X1T = bigp.tile([128, 3, 160], bf16, tag="X1T")
            for mc, (mo, me) in enumerate(CH300):
                ps = psC.tile([me - mo, 160], f32, tag="ps", name="psX1T")
                for k in range(8):
                    nc.tensor.matmul(
                        ps[:], Z[:, k, mo:me], S_bd[:, k, :],
                        start=(k == 0), stop=(k == 7),
                    )
                nc.vector.tensor_copy(X1T[0:me - mo, mc, :], ps[:])

            # T_bd = A0_bd @ S_bd ; T2_bd = A0_bd^T @ S_bd  (block diag)
            T_bd = bigp.tile([128, 8, 160], bf16, tag="T_bd")
            T2_bd = bigp.tile([128, 8, 160], bf16, tag="T2_bd")
            nc.any.memzero(T_bd[:])
            nc.any.memzero(T2_bd[:])
            for c in range(8):
                psT = psC.tile([128, 20], f32, tag="ps", name="psT")
                nc.tensor.matmul(psT[:], adiagT[:, c, :],
                                 S_bd[:, c, c * 20:c * 20 + 20],
                                 start=True, stop=True)
                nc.vector.tensor_copy(T_bd[:, c, c * 20:c * 20 + 20], psT[:])
                psT2 = psC.tile([128, 20], f32, tag="ps", name="psT2")
                nc.tensor.matmul(psT2[:], adiag[:, c, :],
                                 S_bd[:, c, c * 20:c * 20 + 20],
                                 start=True, stop=True)
                nc.vector.tensor_copy(T2_bd[:, c, c * 20:c * 20 + 20], psT2[:])

            # A1_bd = S_bd^T @ T_bd ; A1T_bd = S_bd^T @ T2_bd   [160, 160]
            A1bd = bigp.tile([80, 2, 160], bf16, tag="A1bd")
            A1Tbd = bigp.tile([80, 2, 160], bf16, tag="A1Tbd")
            for mc in range(2):
                ps1 = psC.tile([80, 160], f32, tag="ps", name="psA1")
                ps2 = psC.tile([80, 160], f32, tag="ps", name="psA1T")
                for k in range(8):
                    nc.tensor.matmul(
                        ps1[:], S_bd[:, k, mc * 80:(mc + 1) * 80], T_bd[:, k, :],
                        start=(k == 0), stop=(k == 7))
                for k in range(8):
                    nc.tensor.matmul(
                        ps2[:], S_bd[:, k, mc * 80:(mc + 1) * 80], T2_bd[:, k, :],
                        start=(k == 0), stop=(k == 7))
                nc.vector.tensor_copy(A1bd[:, mc, :], ps1[:])
                nc.vector.tensor_copy(A1Tbd[:, mc, :], ps2[:])

            # ---- level 1 ----
            # Y1pe [160, 450] row-major = X1 @ [pWh1|eWh1]
            Y1pe = bigp.tile([80, 2, 450], bf16, tag="Y1pe")
            for mi in range(2):
                pse = psC.tile([80, 450], f32, tag="ps", name="psY1pe")
                for kc, (ko, ke) in enumerate(CH300):
                    nc.tensor.matmul(
                        pse[:], X1T[0:ke - ko, kc, mi * 80:(mi + 1) * 80],
                        w1cat[0:ke - ko, kc, :],
                        start=(kc == 0), stop=(kc == 2))
                nc.vector.tensor_copy(Y1pe[:, mi, :], pse[:])

            # M1pt [150, 160] = Y1p^T @ A1T_bd, relu -> G1p [75, 2, 160]
            G1p = bigp.tile([75, 2, 160], bf16, tag="G1p")
            for mf in range(2):
                ps = psC.tile([75, 160], f32, tag="ps", name="psM1p")
                for kc in range(2):
                    nc.tensor.matmul(
                        ps[:], Y1pe[:, kc, mf * 75:(mf + 1) * 75],
                        A1Tbd[:, kc, :],
                        start=(kc == 0), stop=(kc == 1))
                nc.scalar.activation(G1p[:, mf, :], ps[:], Relu)

            G1e = bigp.tile([100, 3, 160], bf16, tag="G1e")
            for mf in range(3):
                ps = psC.tile([100, 160], f32, tag="ps", name="psM1e")
                for kc in range(2):
                    nc.tensor.matmul(
                        ps[:], Y1pe[:, kc, 150 + mf * 100:150 + (mf + 1) * 100],
                        A1Tbd[:, kc, :],
                        start=(kc == 0), stop=(kc == 1))
                nc.scalar.activation(G1e[:, mf, :], ps[:], Relu)

            # pool chain level 1
            H1p1 = bigp.tile([100, 3, 160], bf16, tag="H1p1")
            for mc in range(3):
                ps = psC.tile([100, 160], f32, tag="ps", name="psH1p1")
                for kc in range(2):
                    nc.tensor.matmul(
                        ps[:], pWl1[:, kc, mc * 100:(mc + 1) * 100], G1p[:, kc, :],
                        start=(kc == 0), stop=(kc == 1))
                nc.scalar.activation(H1p1[:, mc, :], ps[:], Relu)

            H1e1 = bigp.tile([120, 5, 160], bf16, tag="H1e1")
            for mc in range(5):
                ps = psC.tile([120, 160], f32, tag="ps", name="psH1e1")
                for kc in range(3):
                    nc.tensor.matmul(
                        ps[:], eWl1[:, kc, mc * 120:(mc + 1) * 120], G1e[:, kc, :],
                        start=(kc == 0), stop=(kc == 2))
                nc.scalar.activation(H1e1[:, mc, :], ps[:], Relu)

            S1_bd = bigp.tile([80, 2, 64], bf16, tag="S1_bd")
            s1mask = load(d_s1mask, [80, 2, 64])
            for mi in range(2):
                ps = psC.tile([80, K1], f32, tag="ps", name="psL1")
                for kc in range(3):
                    nc.tensor.matmul(
                        ps[:], H1p1[:, kc, mi * 80:(mi + 1) * 80], pWo1[:, kc, :],
                        start=(kc == 0), stop=(kc == 2))
                s_sb = tmp.tile([80, K1], bf16, tag="s1")
                _softmax_rowmajor(nc, tmp, ps, s_sb, K1)
                # block-diag scatter: replicate the [80,4] softmax 16x along
                # free dim and mask to the owning graph's 4 columns
                nc.vector.tensor_tensor(
                    S1_bd[:, mi, :].rearrange("p (b j) -> p b j", j=K1),
                    s_sb[:, None, :].to_broadcast((80, GPC, K1)),
                    s1mask[:, mi, :].rearrange("p (b j) -> p b j", j=K1),
                    mybir.AluOpType.mult)

            Z1 = bigp.tile([80, 2, 300], bf16, tag="Z1")
            for mi in range(2):
                ps = psC.tile([80, 300], f32, tag="ps", name="psZ1")
                for kc in range(5):
                    nc.tensor.matmul(
                        ps[:], H1e1[:, kc, mi * 80:(mi + 1) * 80], eWo1[:, kc, :],
                        start=(kc == 0), stop=(kc == 4))
                nc.vector.tensor_copy(Z1[:, mi, :], ps[:])

            # pooling level 1
            X2T = bigp.tile([100, 3, 64], bf16, tag="X2T")
            for mc in range(3):
                ps = psC.tile([100, 64], f32, tag="ps", name="psX2T")
                for kc in range(2):
                    nc.tensor.matmul(
                        ps[:], Z1[:, kc, mc * 100:(mc + 1) * 100], S1_bd[:, kc, :],
                        start=(kc == 0), stop=(kc == 1))
                nc.vector.tensor_copy(X2T[:, mc, :], ps[:])

            # T3 = A1_bd^T @ S1_bd ; A2T_bd = S1_bd^T @ T3   [64, 64]
            T3 = bigp.tile([80, 2, 64], bf16, tag="T3")
            for mi in range(2):
                ps = psC.tile([80, 64], f32, tag="ps", name="psT3")
                for kc in range(2):
                    nc.tensor.matmul(
                        ps[:], A1bd[:, kc, mi * 80:(mi + 1) * 80], S1_bd[:, kc, :],
                        start=(kc == 0), stop=(kc == 1))
                nc.vector.tensor_copy(T3[:, mi, :], ps[:])
            A2Tbd = bigp.tile([64, 64], bf16, tag="A2Tbd")
            psA2 = psC.tile([64, 64], f32, tag="ps", name="psA2T")
            for kc in range(2):
                nc.tensor.matmul(
                    psA2[:], S1_bd[:, kc, :], T3[:, kc, :],
                    start=(kc == 0), stop=(kc == 1))
            nc.vector.tensor_copy(A2Tbd[:], psA2[:])

            # ---- level 2 (emb only; S2 == 1) ----
            Y2 = bigp.tile([64, 300], bf16, tag="Y2")
            psY2 = psC.tile([64, 300], f32, tag="ps", name="psY2")
            for kc in range(3):
                nc.tensor.matmul(
                    psY2[:], X2T[:, kc, 0:64], eWh2[:, kc, :],
                    start=(kc == 0), stop=(kc == 2))
            nc.vector.tensor_copy(Y2[:], psY2[:])

            G2 = bigp.tile([100, 3, 64], bf16, tag="G2")
            for mf in range(3):
                ps = psC.tile([100, 64], f32, tag="ps", name="psM2")
                nc.tensor.matmul(
                    ps[:], Y2[:, mf * 100:(mf + 1) * 100], A2Tbd[:],
                    start=True, stop=True)
                nc.scalar.activation(G2[:, mf, :], ps[:], Relu)

            H2 = bigp.tile([120, 5, 64], bf16, tag="H2")
            for mc in range(5):
                ps = psC.tile([120, 64], f32, tag="ps", name="psH2")
                for kc in range(3):
                    nc.tensor.matmul(
                        ps[:], eWl2[:, kc, mc * 120:(mc + 1) * 120], G2[:, kc, :],
                        start=(kc == 0), stop=(kc == 2))
                nc.scalar.activation(H2[:, mc, :], ps[:], Relu)

            Z2 = bigp.tile([64, 300], bf16, tag="Z2")
            psZ2 = psC.tile([64, 300], f32, tag="ps", name="psZ2")
            for kc in range(5):
                nc.tensor.matmul(
                    psZ2[:], H2[:, kc, 0:64], eWo2[:, kc, :],
                    start=(kc == 0), stop=(kc == 4))
            nc.vector.tensor_copy(Z2[:], psZ2[:])

            # X3T [300, 16] = Z2^T @ ones_bd
            X3T = bigp.tile([100, 3, GPC], bf16, tag="X3T")
            for mf in range(3):
                ps = psC.tile([100, GPC], f32, tag="ps", name="psX3T")
                nc.tensor.matmul(
                    ps[:], Z2[:, mf * 100:(mf + 1) * 100], ones16[:],
                    start=True, stop=True)
                nc.vector.tensor_copy(X3T[:, mf, :], ps[:])

            # ---- head ----
            hT = bigp.tile([120, 5, GPC], bf16, tag="hT")
            for mc in range(5):
                ps = psC.tile([120, GPC], f32, tag="ps", name="psh")
                for kc in range(3):
                    nc.tensor.matmul(
                        ps[:], lW1[:, kc, mc * 120:(mc + 1) * 120], X3T[:, kc, :],
                        start=(kc == 0), stop=(kc == 2))
                nc.scalar.activation(hT[:, mc, :], ps[:], Relu,
                                     bias=lb1[:, mc:mc + 1])

            psO = psC.tile([128, GPC], f32, tag="ps", name="psO")
            for kc in range(5):
                nc.tensor.matmul(
                    psO[:], lW2[:, kc, :], hT[:, kc, :],
                    start=(kc == 0), stop=(kc == 4))
            outT = tmp.tile([128, GPC], f32, tag="outT")
            nc.vector.tensor_scalar_add(outT[:], psO[:], lb2[:])
            nc.sync.dma_start(d_out[:], outT[:])

    _split_excess_waits(nc)
    return nc


def _host_prep(inputs):
    """Build per-core in_maps from the full inputs."""
    ONE = np.uint8(0x38)  # 1.0 in float8_e4m3

    x = np.asarray(inputs["x"], np.float32)
    ei = np.asarray(inputs["edge_index"]).astype(np.int64)

    # full A^T in bf16 bit pattern: AT[j, i] = A[i, j]
    ATu = np.zeros((N_NODES, N_NODES), np.uint8)
    ATu[ei[1], ei[0]] = ONE

    # x8[q][p, m, f] = x[128*(8q+m) + p, f], padded 300 -> 304 so the
    # DoubleRow Ko step (304 B) stays 16-byte aligned
    xp = np.zeros((N_NODES, 304), np.float32)
    xp[:, 0:300] = x
    x8 = np.ascontiguousarray(
        xp.reshape(8, 8, 128, 304).transpose(0, 2, 1, 3)).astype(F8)

    def chunkw(w, p, c):
        w = np.asarray(w, np.float32)
        return np.ascontiguousarray(
            w.reshape(c, p, -1).transpose(1, 0, 2)).astype(BF)

    def padchunk(w, rowchunks, c, m):
        w = np.asarray(w, np.float32)
        out = np.zeros((128, c, m), np.float32)
        for ci, (a, b) in enumerate(rowchunks):
            out[0:b - a, ci, :] = w[a:b, :]
        return out.astype(BF)

    wcat0 = np.zeros((300, 492), np.float32)
    wcat0[:, 0:150] = np.asarray(inputs["pWh0"], np.float32)
    wcat0[:, 192:492] = np.asarray(inputs["eWh0"], np.float32)

    ones16 = np.zeros((64, GPC), BF)
    for b in range(GPC):
        ones16[b * 4:(b + 1) * 4, b] = 1
    s1mask = np.zeros((80, 2, 64), BF)
    for mi in range(2):
        for p in range(80):
            gb = (80 * mi + p) // K1NODES
            s1mask[p, mi, gb * 4:(gb + 1) * 4] = 1
    lb1 = np.ascontiguousarray(
        np.asarray(inputs["lb1"], np.float32).reshape(5, 120).T)
    lb2 = np.asarray(inputs["lb2"], np.float32).reshape(128, 1)

    CH300 = [(0, 128), (128, 256), (256, 300)]
    CH600 = [(0, 128), (128, 256), (256, 384), (384, 512), (512, 600)]
    w1cat = np.concatenate([np.asarray(inputs["pWh1"], np.float32),
                            np.asarray(inputs["eWh1"], np.float32)], axis=1)

    shared = {
        "x8": x8,
        "wcat0": chunkw(wcat0, 100, 3),
        "pWl0": padchunk(inputs["pWl0"], [(0, 128), (128, 150)], 2, 300),
        "pWo0": padchunk(inputs["pWo0"], CH300, 3, K0),
        "eWl0": padchunk(inputs["eWl0"], [(0, 64), (64, 192), (192, 300)], 3, 600),
        "eWo0": padchunk(inputs["eWo0"], CH600, 5, 300),
        "w1cat": padchunk(w1cat, CH300, 3, 450),
        "pWl1": chunkw(inputs["pWl1"], 75, 2),
        "pWo1": chunkw(inputs["pWo1"], 100, 3),
        "eWl1": chunkw(inputs["eWl1"], 100, 3),
        "eWo1": chunkw(inputs["eWo1"], 120, 5),
        "eWh2": chunkw(inputs["eWh2"], 100, 3),
        "eWl2": chunkw(inputs["eWl2"], 100, 3),
        "eWo2": chunkw(inputs["eWo2"], 120, 5),
        "lW1": chunkw(inputs["lW1"], 100, 3),
        "lW2": chunkw(inputs["lW2"], 120, 5),
        "lb1": lb1,
        "lb2": lb2,
        "ones16": ones16,
        "s1mask": s1mask,
    }

    in_maps = []
    for d in range(N_CORES):
        r0 = d * R
        slab = ATu[:, r0:r0 + R]  # [8192, 1024]
        at = np.ascontiguousarray(
            slab.reshape(32, 2, 128, 1024).transpose(0, 2, 1, 3)).view(F8)

        adiag = np.zeros((128, 8, 128), np.uint8)
        adiagT = np.zeros((128, 8, 128), np.uint8)
        for c in range(8):
            # full 128x128 slab block, then mask to per-graph 64x64 diag
            blkT = slab[r0 + 128 * c: r0 + 128 * (c + 1),
                        128 * c: 128 * (c + 1)]  # blkT[q, p] = A[rows p, cols q]
            blk = blkT.T
            for h in range(2):
                s = slice(64 * h, 64 * (h + 1))
                adiag[s, c, s] = blk[s, s]
                adiagT[s, c, s] = blkT[s, s]
        m = dict(shared)
        m["at"] = at
        m["adiag"] = adiag.view(F8).astype(BF)
        m["adiagT"] = adiagT.view(F8).astype(BF)
        in_maps.append(m)
    return in_maps


def _run(inputs, trace=False, trace_kwargs=None):
    try:
        import concourse.bass as bass  # noqa: F401
    except ImportError:
        import sys
        sys.path.insert(0, "/opt/trn_rl_repo")
    from concourse.bass_utils import run_bass_kernel_spmd

    if "prog" not in _prog_cache:
        _prog_cache["prog"] = _build_program()
    nc = _prog_cache["prog"]

    in_maps = _host_prep(inputs)
    res = run_bass_kernel_spmd(
        nc, in_maps, core_ids=list(range(N_CORES)), trace=trace,
        **(trace_kwargs or {}),
    )
    out = np.empty((B, 128), np.float32)
    for d in range(N_CORES):
        out[d * GPC:(d + 1) * GPC, :] = res.results[d]["out"].T
    return out, res


def kernel(**inputs):
    out, _ = _run(inputs, trace=False)
    return out

